# revision 1
# baseline (speedup 1.0000x reference)
"""MoE-routed DIAYN discriminator kernel for 8 Trainium2 NeuronCores.

Reference semantics: x = concat([graph, state, next_state], -1); for each
row, run the 3-layer MLP of the LAST factor i<NF with graph[:, i]==1
(rows with no active factor output 0). The dense reference computes all
NF expert MLPs for every row; we instead route each row to exactly one
expert on the host, pack rows into 8 SPMD shards, and run one dense
per-expert MLP stream per core.

Sharding: rows are grouped by expert into BLK-row blocks. Every core
executes the same static "profile" of G runs (run g = prof[g] blocks);
each run uses one weight set, supplied per-core as data. A small host-side
search picks (G, prof) and an assignment of runs -> experts that covers
the actual per-expert block counts with minimal padding + weight traffic.

Device kernel (per run, per block, activations kept transposed [feat, row]):
  h1 = relu(W1^T x + b1); h2 = relu(W2^T h1 + b2); out = W3^T h2 + b3
matmuls run as fp32 bitcast to float32r (full-rate fp32 on the PE).
"""

import numpy as np

import concourse.bass as bass
import concourse.mybir as mybir
from concourse import bacc
from concourse.tile import TileContext
from concourse.bass_utils import run_bass_kernel_spmd

NCORES = 8
BLK = 272  # rows per matmul block; >=256 (f32r full rate), <=512 (PSUM bank)

F32 = mybir.dt.float32
F32R = mybir.dt.float32r

# Rough per-core cost weights for the plan search (ns).
_COST_BLOCK = int(152 * (BLK / 2.4 + 3))  # PE ns per block (152 matmuls)
_COST_RUN = 12_000  # partially-exposed weight-set DMA per extra run

_program_cache = {}


# ---------------------------------------------------------------- planning
def _compositions(total, parts):
    """Non-increasing positive integer compositions of `total` into `parts`."""
    if parts == 1:
        yield (total,)
        return
    for first in range((total + parts - 1) // parts, total - parts + 2):
        for rest in _compositions(total - first, parts - 1):
            if rest[0] <= first:
                yield (first,) + rest


def _try_assign(demands, prof):
    """Greedy cover of per-expert block demands by the 8x-replicated profile.

    demands: list of (n_blocks, expert) sorted desc. Returns dict
    run_size -> list of experts (8 entries per profile slot of that size,
    padding slots filled with the largest expert) or None if infeasible.
    """
    runs = sorted([t for t in prof for _ in range(NCORES)], reverse=True)
    used = []  # (size, expert)
    for n, e in demands:
        rem = n
        while rem > 0:
            if not runs:
                return None
            # largest run <= rem, else smallest run (minimal overshoot)
            pick = None
            for i, s in enumerate(runs):
                if s <= rem:
                    pick = i
                    break
            if pick is None:
                pick = len(runs) - 1
            s = runs.pop(pick)
            used.append((s, e))
            rem -= s
    pad_expert = demands[0][1]
    for s in runs:
        used.append((s, pad_expert))
    by_size = {}
    for s, e in used:
        by_size.setdefault(s, []).append(e)
    return by_size


def _make_plan(nblk):
    """nblk: per-expert block counts. Returns (prof, expert_of[core][g])."""
    demands = sorted(
        [(n, e) for e, n in enumerate(nblk) if n > 0], reverse=True
    )
    total = sum(n for n, _ in demands)
    mincap = (total + NCORES - 1) // NCORES
    best = None
    for G in range(1, 9):
        for cap in range(mincap, mincap + 6):
            for prof in _compositions(cap, G):
                a = _try_assign(demands, prof)
                if a is None:
                    continue
                cost = cap * _COST_BLOCK + G * _COST_RUN
                if best is None or cost < best[0]:
                    best = (cost, prof, a)
    assert best is not None, "no feasible run plan found"
    _, prof, by_size = best
    queues = {s: list(es) for s, es in by_size.items()}
    expert_of = [[None] * len(prof) for _ in range(NCORES)]
    for g, s in enumerate(prof):
        for core in range(NCORES):
            expert_of[core][g] = queues[s].pop(0)
    return list(prof), expert_of


# ---------------------------------------------------------------- device
def _build_program(prof, KO1, KO2, H, C, blk):
    """Build + compile the SPMD Bass program for a run profile."""
    key = (tuple(prof), KO1, KO2, H, C, blk)
    if key in _program_cache:
        return _program_cache[key]

    G = len(prof)
    NB = sum(prof)
    INP = KO1 * 128
    M1 = H // 128
    relu = mybir.ActivationFunctionType.Relu
    ident = mybir.ActivationFunctionType.Identity

    nc = bacc.Bacc("TRN2", target_bir_lowering=False, debug=False,
                   num_devices=NCORES)
    x_d = nc.dram_tensor("xb", [NB, 128, KO1, blk], F32R, kind="ExternalInput").ap()
    w1_d = nc.dram_tensor("w1", [G, 128, KO1, H], F32R, kind="ExternalInput").ap()
    w2_d = nc.dram_tensor("w2", [G, 128, KO2, H], F32R, kind="ExternalInput").ap()
    w3_d = nc.dram_tensor("w3", [G, 128, KO2, C], F32R, kind="ExternalInput").ap()
    b1_d = nc.dram_tensor("b1", [G, H], F32, kind="ExternalInput").ap()
    b2_d = nc.dram_tensor("b2", [G, H], F32, kind="ExternalInput").ap()
    b3_d = nc.dram_tensor("b3", [G, C], F32, kind="ExternalInput").ap()
    out_d = nc.dram_tensor("outb", [NB, C, blk], F32, kind="ExternalOutput").ap()

    runs = []
    for g, T in enumerate(prof):
        runs += [g] * T

    with TileContext(nc) as tc:
        with (
            tc.tile_pool(name="w", bufs=2) as wpool,
            tc.tile_pool(name="x", bufs=2) as xpool,
            tc.tile_pool(name="h1", bufs=3) as h1pool,
            tc.tile_pool(name="h2", bufs=1) as h2pool,
            tc.tile_pool(name="o", bufs=2) as opool,
            tc.tile_pool(name="ps", bufs=8, space="PSUM") as pspool,
        ):
            def emit_weights(g, x_first=None, x_hook=None, x_hook2=None):
                # Biases first (tiny, needed by the first relu). W1 as
                # per-k-tile chunks so block-0's k-outer L1 can consume
                # them as they arrive; W2 as halves (needed later).
                w1ch = []
                b1sb = b2sb = b3sb = None
                for k in range(KO1):
                    if x_first is not None:
                        nc.sync.dma_start(x_first[0][:, k, :],
                                          x_first[1][:, k, :])
                    wt = wpool.tile([128, H], F32R, tag=f"w1k{k}")
                    nc.sync.dma_start(wt[:], w1_d[g, :, k, :])
                    w1ch.append(wt)
                    if k == 0:
                        # Biases after the first chunk pair (PE can start)
                        # but well before the first relu needs them.
                        b1sb = wpool.tile([128, M1], F32, tag="b1")
                        nc.sync.dma_start(
                            b1sb[:],
                            b1_d[g].rearrange("(m p) -> p m", p=128))
                        b2sb = wpool.tile([128, M1], F32, tag="b2")
                        nc.sync.dma_start(
                            b2sb[:],
                            b2_d[g].rearrange("(m p) -> p m", p=128))
                        b3sb = wpool.tile([C, 1], F32, tag="b3")
                        nc.sync.dma_start(b3sb[:], b3_d[g][:, None])
                if x_hook is not None:
                    x_hook()
                KH2 = KO2 // 2
                w2a = wpool.tile([128, KH2, H], F32R, tag="w2a")
                nc.sync.dma_start(w2a[:], w2_d[g, :, :KH2, :])
                if x_hook2 is not None:
                    x_hook2()
                w2b = wpool.tile([128, KO2 - KH2, H], F32R, tag="w2b")
                nc.sync.dma_start(w2b[:], w2_d[g, :, KH2:, :])
                w3sb = wpool.tile([128, KO2, C], F32R, tag="w3")
                nc.sync.dma_start(w3sb[:], w3_d[g])

                def w2(k):
                    return w2a[:, k, :] if k < KH2 else w2b[:, k - KH2, :]

                return dict(w1=lambda k: w1ch[k], w2=w2, w3=w3sb,
                            b1=b1sb, b2=b2sb, b3=b3sb)

            def emit_x(b):
                # x blocks ride the second HWDGE ring (scalar), parallel
                # to the weight stream on sync.
                xsb = xpool.tile([128, KO1, blk], F32R, tag="x")
                nc.scalar.dma_start(xsb[:], x_d[b])
                return xsb

            def emit_L1(W, xsb, kouter=False):
                h1sb = h1pool.tile([128, KO2, blk], F32R, tag="h1")
                if kouter:
                    # All 8 PSUM banks accumulate in parallel; each W1
                    # chunk is fully consumed on arrival (startup mode).
                    pss = [pspool.tile([128, blk], F32, tag="ps",
                                       name=f"ps_ko{m}")
                           for m in range(M1)]
                    for k in range(KO1):
                        for m in range(M1):
                            nc.tensor.matmul(
                                pss[m][:],
                                W["w1"](k)[:, m * 128:(m + 1) * 128],
                                xsb[:, k, :],
                                start=(k == 0), stop=(k == KO1 - 1))
                    for m in range(M1):
                        nc.vector.tensor_scalar(
                            h1sb[:, m, :], pss[m][:], W["b1"][:, m:m + 1],
                            0.0, mybir.AluOpType.add, mybir.AluOpType.max)
                    return h1sb
                for m in range(M1):
                    ps = pspool.tile([128, blk], F32, tag="ps")
                    for k in range(KO1):
                        nc.tensor.matmul(
                            ps[:],
                            W["w1"](k)[:, m * 128:(m + 1) * 128],
                            xsb[:, k, :],
                            start=(k == 0), stop=(k == KO1 - 1))
                    nc.vector.tensor_scalar(
                        h1sb[:, m, :], ps[:], W["b1"][:, m:m + 1], 0.0,
                        mybir.AluOpType.add, mybir.AluOpType.max)
                return h1sb

            def emit_L23(b, W, h1sb):
                h2sb = h2pool.tile([128, KO2, blk], F32R, tag="h2")
                for m in range(M1):
                    ps = pspool.tile([128, blk], F32, tag="ps")
                    for k in range(KO2):
                        nc.tensor.matmul(
                            ps[:],
                            W["w2"](k)[:, m * 128:(m + 1) * 128],
                            h1sb[:, k, :],
                            start=(k == 0), stop=(k == KO2 - 1))
                    nc.scalar.activation(
                        h2sb[:, m, :], ps[:], relu, bias=W["b2"][:, m:m + 1])
                ps3 = pspool.tile([128, blk], F32, tag="ps")
                for k in range(KO2):
                    nc.tensor.matmul(
                        ps3[:C, :],
                        W["w3"][:, k, :],
                        h2sb[:, k, :],
                        start=(k == 0), stop=(k == KO2 - 1))
                osb = opool.tile([C, blk], F32, tag="o")
                nc.scalar.activation(
                    osb[:], ps3[:C, :], ident, bias=W["b3"][:, 0:1])
                nc.gpsimd.dma_start(out_d[b], osb[:])

            # Software pipeline, depth 2: L1 of blocks b+1/b+2 are
            # emitted before L2/L3 of block b, so weight-set DMAs and
            # ACT latency never drain the PE (esp. during the initial
            # HBM-bound weight load).
            Ws = {}
            h1 = {}

            xpre = {}

            def emit_front(b):
                g = runs[b]
                if g not in Ws:
                    Ws[g] = emit_weights(g)
                h1[b] = emit_L1(Ws[g], xpre.pop(b) if b in xpre
                                else emit_x(b))

            # Startup: x0/x1 lead the scalar ring while weights
            # stream on sync; L1(0)/L1(1) are emitted before L2(0) so
            # the PE has work during the HBM-bound weight load. Steady
            # state keeps L1 two blocks ahead of L2/L3.
            def emit_x_sync(b):
                xsb = xpool.tile([128, KO1, blk], F32R, tag="x")
                nc.sync.dma_start(xsb[:], x_d[b])
                return xsb

            # Startup: everything for the first ~3 blocks rides the sync
            # ring in consumption order (x0 interleaved with W1 chunks,
            # then x1, W2a, x2, W2b); block 0's L1 runs k-outer so each
            # W1 chunk is consumed on arrival.
            g0 = runs[0]
            if prof[0] >= 3:
                xsb0 = xpool.tile([128, KO1, blk], F32R, tag="x")
                xs = {}
                def _x12():
                    xs[1] = emit_x_sync(1)
                    xs[2] = emit_x_sync(2)

                Ws[g0] = emit_weights(g0, x_first=(xsb0, x_d[0]),
                                      x_hook=_x12)
                # x3/x4 ride the idle SWDGE ring: the scalar ring's
                # issue slot is blocked behind early L2-relus right at
                # the prologue->steady transition.
                for bb in (3, 4):
                    if bb < NB:
                        xp = xpool.tile([128, KO1, blk], F32R, tag="x",
                                        name=f"xpre{bb}")
                        nc.gpsimd.dma_start(xp[:], x_d[bb])
                        xpre[bb] = xp
                h1[0] = emit_L1(Ws[g0], xsb0, kouter=True)
                h1[1] = emit_L1(Ws[g0], xs[1])
                h1[2] = emit_L1(Ws[g0], xs[2])
                emitted = 2
            elif NB > 1 and runs[1] == g0:
                xsb0 = xpool.tile([128, KO1, blk], F32R, tag="x")
                xs1 = []
                Ws[g0] = emit_weights(g0, x_first=(xsb0, x_d[0]),
                                      x_hook=lambda: xs1.append(emit_x(1)))
                h1[0] = emit_L1(Ws[g0], xsb0, kouter=True)
                h1[1] = emit_L1(Ws[g0], xs1[0])
                emitted = 1
            else:
                emit_front(0)
                emitted = 0
            for b in range(NB):
                for nxt in range(emitted + 1, min(b + 3, NB)):
                    emit_front(nxt)
                    emitted = nxt
                if b + 4 < NB and runs[b + 4] not in Ws:
                    Ws[runs[b + 4]] = emit_weights(runs[b + 4])
                emit_L23(b, Ws[runs[b]], h1.pop(b))

    nc.compile()
    _program_cache[key] = nc
    return nc


# ---------------------------------------------------------------- host
def _execute(inputs, trace=False, trace_cores=None):
    graph = np.ascontiguousarray(inputs["graph"], dtype=np.float32)
    state = np.ascontiguousarray(inputs["state"], dtype=np.float32)
    next_state = np.ascontiguousarray(inputs["next_state"], dtype=np.float32)
    W1 = np.ascontiguousarray(inputs["W1"], dtype=np.float32)
    b1 = np.ascontiguousarray(inputs["b1"], dtype=np.float32)
    W2 = np.ascontiguousarray(inputs["W2"], dtype=np.float32)
    b2 = np.ascontiguousarray(inputs["b2"], dtype=np.float32)
    W3 = np.ascontiguousarray(inputs["W3"], dtype=np.float32)
    b3 = np.ascontiguousarray(inputs["b3"], dtype=np.float32)

    B = graph.shape[0]
    NF, IN, H = W1.shape
    C = W3.shape[2]
    assert IN == graph.shape[1] + state.shape[1] + next_state.shape[1]
    assert H % 128 == 0 and C <= 128
    INP = ((IN + 127) // 128) * 128
    KO1 = INP // 128

    out_full = np.zeros((B, C), dtype=np.float32)

    # --- route: last active factor per row
    mask = graph[:, :NF] == 1.0
    active = mask.any(axis=1)
    last = (NF - 1) - np.argmax(mask[:, ::-1], axis=1)
    if not active.any():
        return (out_full, None) if trace else out_full

    rows_by_e = [np.nonzero(active & (last == e))[0] for e in range(NF)]
    nblk = [(len(r) + BLK - 1) // BLK for r in rows_by_e]
    prof, expert_of = _make_plan(nblk)
    G, NB = len(prof), sum(prof)

    # --- pack rows into per-core block slots
    # rowmap[core] : int32 [NB, BLK], original row id or -1 (pad)
    rowmap = [np.full((NB, BLK), -1, dtype=np.int64) for _ in range(NCORES)]
    off = np.cumsum([0] + prof)  # run g occupies blocks [off[g], off[g+1])
    slots_by_e = {}
    for core in range(NCORES):
        for g in range(G):
            slots_by_e.setdefault(expert_of[core][g], []).append((core, g))
    for e in range(NF):
        rows = rows_by_e[e]
        if len(rows) == 0:
            continue
        pos = 0
        for core, g in slots_by_e.get(e, []):
            cap = prof[g] * BLK
            take = min(cap, len(rows) - pos)
            if take <= 0:
                break
            flat = rowmap[core][off[g]:off[g + 1]].reshape(-1)
            flat[:take] = rows[pos:pos + take]
            pos += take
        assert pos == len(rows), f"expert {e} rows not fully packed"

    # --- build per-core inputs
    x = np.concatenate([graph, state, next_state], axis=1)  # [B, IN]
    if INP != IN:
        x = np.concatenate([x, np.zeros((B, INP - IN), np.float32)], axis=1)
    xpad = np.concatenate([x, np.zeros((1, INP), np.float32)], axis=0)
    W1p = np.zeros((NF, INP, H), np.float32)
    W1p[:, :IN] = W1

    # Partition-major device layouts: [.., 128, KO, free] so every DMA
    # line is one contiguous 10-20KB run per partition.
    KO2 = H // 128
    W1pm = np.ascontiguousarray(
        W1p.reshape(NF, KO1, 128, H).transpose(0, 2, 1, 3))
    W2pm = np.ascontiguousarray(
        W2.reshape(NF, KO2, 128, H).transpose(0, 2, 1, 3))
    W3pm = np.ascontiguousarray(
        W3.reshape(NF, KO2, 128, C).transpose(0, 2, 1, 3))
    in_maps = []
    for core in range(NCORES):
        xb = xpad[rowmap[core].reshape(-1)]  # [NB*BLK, INP]; -1 -> zero row
        xb = np.ascontiguousarray(
            xb.reshape(NB, BLK, KO1, 128).transpose(0, 3, 2, 1))
        es = expert_of[core]
        in_maps.append({
            "xb": xb,
            "w1": W1pm[es],
            "w2": W2pm[es],
            "w3": W3pm[es],
            "b1": np.ascontiguousarray(b1[es]),
            "b2": np.ascontiguousarray(b2[es]),
            "b3": np.ascontiguousarray(b3[es]),
        })

    nc = _build_program(prof, KO1, KO2, H, C, BLK)
    kwargs = {}
    if trace:
        kwargs = dict(trace=True,
                      trace_cores=trace_cores or list(range(NCORES)))
    res = run_bass_kernel_spmd(nc, in_maps, list(range(NCORES)), **kwargs)

    # --- scatter back
    for core in range(NCORES):
        ob = np.asarray(res.results[core]["outb"])  # [NB, C, BLK]
        rows = ob.transpose(0, 2, 1).reshape(NB * BLK, C)
        ids = rowmap[core].reshape(-1)
        valid = ids >= 0
        out_full[ids[valid]] = rows[valid]

    return (out_full, res) if trace else out_full


def kernel(**inputs):
    return _execute(inputs)



# revision 12
# speedup vs baseline: 1.0237x; 1.0237x over previous
"""MoE-routed DIAYN discriminator kernel for 8 Trainium2 NeuronCores.

Reference semantics: x = concat([graph, state, next_state], -1); for each
row, run the 3-layer MLP of the LAST factor i<NF with graph[:, i]==1
(rows with no active factor output 0). The dense reference computes all
NF expert MLPs for every row; we instead route each row to exactly one
expert on the host, pack rows into 8 SPMD shards, and run one dense
per-expert MLP stream per core.

Sharding: rows are grouped by expert into BLK-row blocks. Every core
executes the same static "profile" of G runs (run g = prof[g] blocks);
each run uses one weight set, supplied per-core as data. A small host-side
search picks (G, prof) and an assignment of runs -> experts that covers
the actual per-expert block counts with minimal padding + weight traffic.

Device kernel (per run, per block, activations kept transposed [feat, row]):
  h1 = relu(W1^T x + b1); h2 = relu(W2^T h1 + b2); out = W3^T h2 + b3
matmuls run in bf16 (full rate on the PE, half the DMA of fp32);
PSUM accumulation, biases and the final output stay fp32.
"""

import ml_dtypes
import numpy as np

import concourse.bass as bass
import concourse.mybir as mybir
from concourse import bacc
from concourse.tile import TileContext
from concourse.bass_utils import run_bass_kernel_spmd

NCORES = 8
BLK = 272  # rows per matmul block; <=512 (PSUM bank)

F32 = mybir.dt.float32
BF16 = mybir.dt.bfloat16
NP_BF16 = ml_dtypes.bfloat16

# Rough per-core cost weights for the plan search (ns).
_COST_BLOCK = int(152 * (BLK / 2.4 + 3))  # PE ns per block (152 matmuls)
_COST_RUN = 12_000  # partially-exposed weight-set DMA per extra run

# v2 (variable-block) cost weights, from HW measurement: a bf16 matmul of
# N columns takes ~N/2.4 + 3 ns, 152 matmuls per block; a per-block
# weight-set DMA adds queue pressure and the set for block b prefetches
# during block b-2/b-1's compute.
_V2_NS_PER_ROW = 152 / 2.4
_V2_NS_PER_BLOCK = 152 * 3 + 1200
_V2_SET_DMA_NS = 13_000  # per-ring half weight set at ~200 GB/s

_program_cache = {}


# ---------------------------------------------------------------- planning
def _compositions(total, parts):
    """Non-increasing positive integer compositions of `total` into `parts`."""
    if parts == 1:
        yield (total,)
        return
    for first in range((total + parts - 1) // parts, total - parts + 2):
        for rest in _compositions(total - first, parts - 1):
            if rest[0] <= first:
                yield (first,) + rest


def _try_assign(demands, prof):
    """Greedy cover of per-expert block demands by the 8x-replicated profile.

    demands: list of (n_blocks, expert) sorted desc. Returns dict
    run_size -> list of experts (8 entries per profile slot of that size,
    padding slots filled with the largest expert) or None if infeasible.
    """
    runs = sorted([t for t in prof for _ in range(NCORES)], reverse=True)
    used = []  # (size, expert)
    for n, e in demands:
        rem = n
        while rem > 0:
            if not runs:
                return None
            # largest run <= rem, else smallest run (minimal overshoot)
            pick = None
            for i, s in enumerate(runs):
                if s <= rem:
                    pick = i
                    break
            if pick is None:
                pick = len(runs) - 1
            s = runs.pop(pick)
            used.append((s, e))
            rem -= s
    pad_expert = demands[0][1]
    for s in runs:
        used.append((s, pad_expert))
    by_size = {}
    for s, e in used:
        by_size.setdefault(s, []).append(e)
    return by_size


def _make_plan(nblk):
    """nblk: per-expert block counts. Returns (prof, expert_of[core][g])."""
    demands = sorted(
        [(n, e) for e, n in enumerate(nblk) if n > 0], reverse=True
    )
    total = sum(n for n, _ in demands)
    mincap = (total + NCORES - 1) // NCORES
    best = None
    for G in range(1, 9):
        for cap in range(mincap, mincap + 6):
            for prof in _compositions(cap, G):
                a = _try_assign(demands, prof)
                if a is None:
                    continue
                cost = cap * _COST_BLOCK + G * _COST_RUN
                if best is None or cost < best[0]:
                    best = (cost, prof, a)
    assert best is not None, "no feasible run plan found"
    _, prof, by_size = best
    queues = {s: list(es) for s, es in by_size.items()}
    expert_of = [[None] * len(prof) for _ in range(NCORES)]
    for g, s in enumerate(prof):
        for core in range(NCORES):
            expert_of[core][g] = queues[s].pop(0)
    return list(prof), expert_of


# ------------------------------------------------------------- planning v2
def _best_cover(rem, sizes, inv, limit):
    """Min-overshoot multiset of pieces covering `rem` rows.

    sizes: piece sizes desc; inv: available count per size; limit: prune
    bound on overshoot. Returns (overshoot, npieces, counts) or None.
    """
    best = [None]

    def rec(i, need, counts, used):
        if need <= 0:
            os = -need
            cand = (os, used, tuple(counts))
            if best[0] is None or cand < best[0]:
                best[0] = cand
            return
        if i == len(sizes):
            return
        if best[0] is not None and best[0][0] == 0 and used >= best[0][1]:
            return
        s = sizes[i]
        hi = min(inv[s], -(-need // s))
        for n in range(hi, -1, -1):
            if best[0] is not None and n * s - need > best[0][0] >= 0:
                continue
            counts.append(n)
            rec(i + 1, need - n * s, counts, used + n)
            counts.pop()

    rec(0, rem, [], 0)
    if best[0] is None or best[0][0] > limit:
        return None
    return best[0]


def _assign_rows(demands, bsizes):
    """Assign per-expert row demands to the 8x-replicated block slots.

    demands: [(rows, expert)] ascending. bsizes: per-core block sizes.
    Returns {expert: [(size, rows_used), ...]} or None.
    """
    sizes = sorted(set(bsizes), reverse=True)
    inv = {s: 8 * bsizes.count(s) for s in sizes}
    slack = 8 * sum(bsizes) - sum(r for r, _ in demands)
    out = {}
    for r, e in demands:
        got = _best_cover(r, sizes, inv, slack)
        if got is None:
            return None
        os, _, counts = got
        slack -= os
        pieces = []
        rem = r
        for s, n in zip(sizes, counts):
            inv[s] -= n
            for _ in range(n):
                take = min(rem, s)
                pieces.append((s, take))
                rem -= take
        out[e] = pieces
    return out


def _plan_blocks(rows):
    """Pick per-core block sizes + slot->expert map for realized row counts.

    Returns (bsizes, expert_of[core][b], nrows_of[core][b]) or None.
    bsizes is shared by all cores (one compiled program); each block slot
    carries its own expert weight set as per-core data.
    """
    demands = sorted((r, e) for e, r in enumerate(rows) if r > 0)
    total = sum(r for r, _ in demands)
    if total == 0:
        return None

    tail_sizes = list(range(512, 31, -32))
    cands = set()
    for nbig in (3, 4, 5):
        base = (512,) * nbig
        cands.add(base)
        for a in tail_sizes:
            cands.add(tuple(sorted(base + (a,), reverse=True)))
            for b in tail_sizes:
                if b <= a:
                    cands.add(tuple(sorted(base + (a, b), reverse=True)))

    def cost(bs):
        c = sum(bs) * _V2_NS_PER_ROW + len(bs) * _V2_NS_PER_BLOCK
        for i in range(1, len(bs)):
            c += max(0.0, _V2_SET_DMA_NS - bs[i - 1] * _V2_NS_PER_ROW)
        return c

    best = None
    for bs in sorted(cands, key=cost):
        cap = 8 * sum(bs)
        if not (total <= cap <= total + 4096) or len(bs) < 3:
            continue
        if best is not None and cost(bs) >= best[0]:
            continue
        assign = _assign_rows(demands, list(bs))
        if assign is None:
            assign = _assign_rows(demands[::-1], list(bs))
        if assign is None:
            continue
        best = (cost(bs), bs, assign)
    if best is None:
        return None
    _, bsizes, assign = best

    # queue of (expert, rows) pieces per size; deal slot-major to cores
    queues = {}
    for e, pieces in assign.items():
        for s, take in pieces:
            queues.setdefault(s, []).append((e, take))
    NB = len(bsizes)
    expert_of = [[0] * NB for _ in range(NCORES)]
    nrows_of = [[0] * NB for _ in range(NCORES)]
    for b, s in enumerate(bsizes):
        for core in range(NCORES):
            q = queues.get(s) or []
            if q:
                e, take = q.pop(0)
            else:
                e, take = 0, 0
            expert_of[core][b] = e
            nrows_of[core][b] = take
    return list(bsizes), expert_of, nrows_of


# ---------------------------------------------------------------- device
def _build_program(prof, KO1, KO2, H, C, blk):
    """Build + compile the SPMD Bass program for a run profile."""
    key = (tuple(prof), KO1, KO2, H, C, blk)
    if key in _program_cache:
        return _program_cache[key]

    G = len(prof)
    NB = sum(prof)
    INP = KO1 * 128
    M1 = H // 128
    relu = mybir.ActivationFunctionType.Relu
    ident = mybir.ActivationFunctionType.Identity

    nc = bacc.Bacc("TRN2", target_bir_lowering=False, debug=False,
                   num_devices=NCORES)
    x_d = nc.dram_tensor("xb", [NB, 128, KO1, blk], BF16, kind="ExternalInput").ap()
    w1_d = nc.dram_tensor("w1", [G, 128, KO1, H], BF16, kind="ExternalInput").ap()
    w2_d = nc.dram_tensor("w2", [G, 128, KO2, H], BF16, kind="ExternalInput").ap()
    w3_d = nc.dram_tensor("w3", [G, 128, KO2, C], BF16, kind="ExternalInput").ap()
    b1_d = nc.dram_tensor("b1", [G, H], F32, kind="ExternalInput").ap()
    b2_d = nc.dram_tensor("b2", [G, H], F32, kind="ExternalInput").ap()
    b3_d = nc.dram_tensor("b3", [G, C], F32, kind="ExternalInput").ap()
    out_d = nc.dram_tensor("outb", [NB, C, blk], F32, kind="ExternalOutput").ap()

    runs = []
    for g, T in enumerate(prof):
        runs += [g] * T

    with TileContext(nc) as tc:
        with (
            tc.tile_pool(name="w", bufs=2) as wpool,
            tc.tile_pool(name="x", bufs=2) as xpool,
            tc.tile_pool(name="h1", bufs=3) as h1pool,
            tc.tile_pool(name="h2", bufs=1) as h2pool,
            tc.tile_pool(name="o", bufs=2) as opool,
            tc.tile_pool(name="ps", bufs=8, space="PSUM") as pspool,
        ):
            def emit_weights(g, x_first=None, x_hook=None, x_hook2=None):
                # Biases first (tiny, needed by the first relu). W1 as
                # per-k-tile chunks so block-0's k-outer L1 can consume
                # them as they arrive; W2 as halves (needed later).
                w1ch = []
                b1sb = b2sb = b3sb = None
                for k in range(KO1):
                    if x_first is not None:
                        nc.sync.dma_start(x_first[0][:, k, :],
                                          x_first[1][:, k, :])
                    wt = wpool.tile([128, H], BF16, tag=f"w1k{k}")
                    nc.sync.dma_start(wt[:], w1_d[g, :, k, :])
                    w1ch.append(wt)
                    if k == 0:
                        # Biases after the first chunk pair (PE can start)
                        # but well before the first relu needs them.
                        b1sb = wpool.tile([128, M1], F32, tag="b1")
                        nc.sync.dma_start(
                            b1sb[:],
                            b1_d[g].rearrange("(m p) -> p m", p=128))
                        b2sb = wpool.tile([128, M1], F32, tag="b2")
                        nc.sync.dma_start(
                            b2sb[:],
                            b2_d[g].rearrange("(m p) -> p m", p=128))
                        b3sb = wpool.tile([C, 1], F32, tag="b3")
                        nc.sync.dma_start(b3sb[:], b3_d[g][:, None])
                if x_hook is not None:
                    x_hook()
                KH2 = KO2 // 2
                w2a = wpool.tile([128, KH2, H], BF16, tag="w2a")
                nc.sync.dma_start(w2a[:], w2_d[g, :, :KH2, :])
                if x_hook2 is not None:
                    x_hook2()
                w2b = wpool.tile([128, KO2 - KH2, H], BF16, tag="w2b")
                nc.sync.dma_start(w2b[:], w2_d[g, :, KH2:, :])
                w3sb = wpool.tile([128, KO2, C], BF16, tag="w3")
                nc.sync.dma_start(w3sb[:], w3_d[g])

                def w2(k):
                    return w2a[:, k, :] if k < KH2 else w2b[:, k - KH2, :]

                return dict(w1=lambda k: w1ch[k], w2=w2, w3=w3sb,
                            b1=b1sb, b2=b2sb, b3=b3sb)

            def emit_x(b):
                # x blocks ride the second HWDGE ring (scalar), parallel
                # to the weight stream on sync.
                xsb = xpool.tile([128, KO1, blk], BF16, tag="x")
                nc.scalar.dma_start(xsb[:], x_d[b])
                return xsb

            def emit_L1(W, xsb, kouter=False):
                h1sb = h1pool.tile([128, KO2, blk], BF16, tag="h1")
                if kouter:
                    # All 8 PSUM banks accumulate in parallel; each W1
                    # chunk is fully consumed on arrival (startup mode).
                    pss = [pspool.tile([128, blk], F32, tag="ps",
                                       name=f"ps_ko{m}")
                           for m in range(M1)]
                    for k in range(KO1):
                        for m in range(M1):
                            nc.tensor.matmul(
                                pss[m][:],
                                W["w1"](k)[:, m * 128:(m + 1) * 128],
                                xsb[:, k, :],
                                start=(k == 0), stop=(k == KO1 - 1))
                    for m in range(M1):
                        nc.vector.tensor_scalar(
                            h1sb[:, m, :], pss[m][:], W["b1"][:, m:m + 1],
                            0.0, mybir.AluOpType.add, mybir.AluOpType.max)
                    return h1sb
                for m in range(M1):
                    ps = pspool.tile([128, blk], F32, tag="ps")
                    for k in range(KO1):
                        nc.tensor.matmul(
                            ps[:],
                            W["w1"](k)[:, m * 128:(m + 1) * 128],
                            xsb[:, k, :],
                            start=(k == 0), stop=(k == KO1 - 1))
                    nc.vector.tensor_scalar(
                        h1sb[:, m, :], ps[:], W["b1"][:, m:m + 1], 0.0,
                        mybir.AluOpType.add, mybir.AluOpType.max)
                return h1sb

            def emit_L23(b, W, h1sb):
                h2sb = h2pool.tile([128, KO2, blk], BF16, tag="h2")
                for m in range(M1):
                    ps = pspool.tile([128, blk], F32, tag="ps")
                    for k in range(KO2):
                        nc.tensor.matmul(
                            ps[:],
                            W["w2"](k)[:, m * 128:(m + 1) * 128],
                            h1sb[:, k, :],
                            start=(k == 0), stop=(k == KO2 - 1))
                    nc.scalar.activation(
                        h2sb[:, m, :], ps[:], relu, bias=W["b2"][:, m:m + 1])
                ps3 = pspool.tile([128, blk], F32, tag="ps")
                for k in range(KO2):
                    nc.tensor.matmul(
                        ps3[:C, :],
                        W["w3"][:, k, :],
                        h2sb[:, k, :],
                        start=(k == 0), stop=(k == KO2 - 1))
                osb = opool.tile([C, blk], F32, tag="o")
                nc.scalar.activation(
                    osb[:], ps3[:C, :], ident, bias=W["b3"][:, 0:1])
                nc.gpsimd.dma_start(out_d[b], osb[:])

            # Software pipeline, depth 2: L1 of blocks b+1/b+2 are
            # emitted before L2/L3 of block b, so weight-set DMAs and
            # ACT latency never drain the PE (esp. during the initial
            # HBM-bound weight load).
            Ws = {}
            h1 = {}

            xpre = {}

            def emit_front(b):
                g = runs[b]
                if g not in Ws:
                    Ws[g] = emit_weights(g)
                h1[b] = emit_L1(Ws[g], xpre.pop(b) if b in xpre
                                else emit_x(b))

            # Startup: x0/x1 lead the scalar ring while weights
            # stream on sync; L1(0)/L1(1) are emitted before L2(0) so
            # the PE has work during the HBM-bound weight load. Steady
            # state keeps L1 two blocks ahead of L2/L3.
            def emit_x_sync(b):
                xsb = xpool.tile([128, KO1, blk], BF16, tag="x")
                nc.sync.dma_start(xsb[:], x_d[b])
                return xsb

            # Startup: everything for the first ~3 blocks rides the sync
            # ring in consumption order (x0 interleaved with W1 chunks,
            # then x1, W2a, x2, W2b); block 0's L1 runs k-outer so each
            # W1 chunk is consumed on arrival.
            g0 = runs[0]
            if prof[0] >= 3:
                xsb0 = xpool.tile([128, KO1, blk], BF16, tag="x")
                xs = {}
                def _x12():
                    xs[1] = emit_x_sync(1)
                    xs[2] = emit_x_sync(2)

                Ws[g0] = emit_weights(g0, x_first=(xsb0, x_d[0]),
                                      x_hook=_x12)
                # x3/x4 ride the idle SWDGE ring: the scalar ring's
                # issue slot is blocked behind early L2-relus right at
                # the prologue->steady transition.
                for bb in (3, 4):
                    if bb < NB:
                        xp = xpool.tile([128, KO1, blk], BF16, tag="x",
                                        name=f"xpre{bb}")
                        nc.gpsimd.dma_start(xp[:], x_d[bb])
                        xpre[bb] = xp
                h1[0] = emit_L1(Ws[g0], xsb0, kouter=True)
                h1[1] = emit_L1(Ws[g0], xs[1])
                h1[2] = emit_L1(Ws[g0], xs[2])
                emitted = 2
            elif NB > 1 and runs[1] == g0:
                xsb0 = xpool.tile([128, KO1, blk], BF16, tag="x")
                xs1 = []
                Ws[g0] = emit_weights(g0, x_first=(xsb0, x_d[0]),
                                      x_hook=lambda: xs1.append(emit_x(1)))
                h1[0] = emit_L1(Ws[g0], xsb0, kouter=True)
                h1[1] = emit_L1(Ws[g0], xs1[0])
                emitted = 1
            else:
                emit_front(0)
                emitted = 0
            for b in range(NB):
                for nxt in range(emitted + 1, min(b + 3, NB)):
                    emit_front(nxt)
                    emitted = nxt
                if b + 4 < NB and runs[b + 4] not in Ws:
                    Ws[runs[b + 4]] = emit_weights(runs[b + 4])
                emit_L23(b, Ws[runs[b]], h1.pop(b))

    nc.compile()
    _program_cache[key] = nc
    return nc


# ---------------------------------------------------------------- device v2
def _build_program2(blocks, KO1, KO2, H, C):
    """Variable-block SPMD program; every block slot has its own weight set
    (per-core data), sizes shared across cores. Weights ride two DMA rings
    (W1+biases on sync, W2/W3 on vector), x on scalar, outputs on gpsimd."""
    key = ("v2", tuple(blocks), KO1, KO2, H, C)
    if key in _program_cache:
        return _program_cache[key]

    NB = len(blocks)
    M1 = H // 128
    KH2 = KO2 // 2
    relu = mybir.ActivationFunctionType.Relu
    ident = mybir.ActivationFunctionType.Identity

    nc = bacc.Bacc("TRN2", target_bir_lowering=False, debug=False,
                   num_devices=NCORES)
    x_ds = [nc.dram_tensor(f"xb{b}", [128, KO1, bs], BF16,
                           kind="ExternalInput").ap()
            for b, bs in enumerate(blocks)]
    out_ds = [nc.dram_tensor(f"outb{b}", [C, bs], F32,
                             kind="ExternalOutput").ap()
              for b, bs in enumerate(blocks)]
    w1_d = nc.dram_tensor("w1", [NB, 128, KO1, H], BF16,
                          kind="ExternalInput").ap()
    w2_d = nc.dram_tensor("w2", [NB, 128, KO2, H], BF16,
                          kind="ExternalInput").ap()
    w3_d = nc.dram_tensor("w3", [NB, 128, KO2, C], BF16,
                          kind="ExternalInput").ap()
    b1_d = nc.dram_tensor("b1", [NB, H], F32, kind="ExternalInput").ap()
    b2_d = nc.dram_tensor("b2", [NB, H], F32, kind="ExternalInput").ap()
    b3_d = nc.dram_tensor("b3", [NB, C], F32, kind="ExternalInput").ap()

    with TileContext(nc) as tc:
        with (
            tc.tile_pool(name="w", bufs=2) as wpool,
            tc.tile_pool(name="x", bufs=2) as xpool,
            tc.tile_pool(name="h1", bufs=3) as h1pool,
            tc.tile_pool(name="h2", bufs=1) as h2pool,
            tc.tile_pool(name="o", bufs=2) as opool,
            tc.tile_pool(name="ps", bufs=8, space="PSUM") as pspool,
        ):
            def emit_weights(b):
                # W1 as per-k chunks + biases on the sync ring; W2 halves
                # and W3 on the scalar ring (bufs=3 so the slot-free waits
                # are already resolved at issue time and never park the
                # scalar engine between PSUM-draining relus).
                w1ch = []
                b1sb = b2sb = b3sb = None
                for k in range(KO1):
                    wt = wpool.tile([128, H], BF16, tag=f"w1k{k}")
                    nc.sync.dma_start(wt[:], w1_d[b, :, k, :])
                    w1ch.append(wt)
                    if k == 0:
                        b1sb = wpool.tile([128, M1], F32, tag="b1", bufs=3)
                        nc.sync.dma_start(
                            b1sb[:],
                            b1_d[b].rearrange("(m p) -> p m", p=128))
                        b2sb = wpool.tile([128, M1], F32, tag="b2", bufs=3)
                        nc.sync.dma_start(
                            b2sb[:],
                            b2_d[b].rearrange("(m p) -> p m", p=128))
                        b3sb = wpool.tile([C, 1], F32, tag="b3", bufs=3)
                        nc.sync.dma_start(b3sb[:], b3_d[b][:, None])
                w2a = wpool.tile([128, KH2, H], BF16, tag="w2a", bufs=3)
                nc.scalar.dma_start(w2a[:], w2_d[b, :, :KH2, :])
                w2b = wpool.tile([128, KO2 - KH2, H], BF16, tag="w2b",
                                 bufs=3)
                nc.scalar.dma_start(w2b[:], w2_d[b, :, KH2:, :])
                w3sb = wpool.tile([128, KO2, C], BF16, tag="w3", bufs=3)
                nc.scalar.dma_start(w3sb[:], w3_d[b])

                def w2(k):
                    return w2a[:, k, :] if k < KH2 else w2b[:, k - KH2, :]

                return dict(w1=lambda k: w1ch[k], w2=w2, w3=w3sb,
                            b1=b1sb, b2=b2sb, b3=b3sb)

            def emit_x(b, ring=None):
                bs = blocks[b]
                xsb = xpool.tile([128, KO1, bs], BF16, tag="x")
                (ring or nc.scalar).dma_start(xsb[:], x_ds[b][:])
                return xsb

            def emit_L1(b, W, xsb, kouter=False):
                bs = blocks[b]
                h1sb = h1pool.tile([128, KO2, bs], BF16, tag="h1")
                if kouter:
                    # Startup: all 8 PSUM banks accumulate in parallel so
                    # each W1 chunk is fully consumed as it arrives.
                    pss = [pspool.tile([128, bs], F32, tag="ps",
                                       name=f"ps_ko{m}")
                           for m in range(M1)]
                    for k in range(KO1):
                        for m in range(M1):
                            nc.tensor.matmul(
                                pss[m][:],
                                W["w1"](k)[:, m * 128:(m + 1) * 128],
                                xsb[:, k, :],
                                start=(k == 0), stop=(k == KO1 - 1))
                    for m in range(M1):
                        nc.vector.tensor_scalar(
                            h1sb[:, m, :], pss[m][:], W["b1"][:, m:m + 1],
                            0.0, mybir.AluOpType.add, mybir.AluOpType.max)
                    return h1sb
                for m in range(M1):
                    ps = pspool.tile([128, bs], F32, tag="ps")
                    for k in range(KO1):
                        nc.tensor.matmul(
                            ps[:],
                            W["w1"](k)[:, m * 128:(m + 1) * 128],
                            xsb[:, k, :],
                            start=(k == 0), stop=(k == KO1 - 1))
                    nc.vector.tensor_scalar(
                        h1sb[:, m, :], ps[:], W["b1"][:, m:m + 1], 0.0,
                        mybir.AluOpType.add, mybir.AluOpType.max)
                return h1sb

            def emit_L23(b, W, h1sb):
                bs = blocks[b]
                h2sb = h2pool.tile([128, KO2, bs], BF16, tag="h2")
                for m in range(M1):
                    ps = pspool.tile([128, bs], F32, tag="ps")
                    for k in range(KO2):
                        nc.tensor.matmul(
                            ps[:],
                            W["w2"](k)[:, m * 128:(m + 1) * 128],
                            h1sb[:, k, :],
                            start=(k == 0), stop=(k == KO2 - 1))
                    nc.scalar.activation(
                        h2sb[:, m, :], ps[:], relu, bias=W["b2"][:, m:m + 1])
                ps3 = pspool.tile([128, bs], F32, tag="ps")
                for k in range(KO2):
                    nc.tensor.matmul(
                        ps3[:C, :],
                        W["w3"][:, k, :],
                        h2sb[:, k, :],
                        start=(k == 0), stop=(k == KO2 - 1))
                osb = opool.tile([C, bs], F32, tag="o")
                nc.scalar.activation(
                    osb[:], ps3[:C, :], ident, bias=W["b3"][:, 0:1])
                nc.gpsimd.dma_start(out_ds[b][:], osb[:])

            # Two-deep software pipeline: L1 leads L23 by two blocks so the
            # per-block weight-set DMA always overlaps prior compute.
            # Startup: x0 leads the scalar ring (ahead of W2/W3 of block
            # 0), x1 rides sync behind W1(0)'s chunks, and block 0 runs
            # k-outer so W1 chunks are consumed as they arrive.
            h1 = {}
            Ws = {}
            xsb0 = emit_x(0)
            Ws[0] = emit_weights(0)
            if NB > 1:
                x1 = emit_x(1, ring=nc.sync)
            h1[0] = emit_L1(0, Ws[0], xsb0, kouter=True)
            if NB > 1:
                Ws[1] = emit_weights(1)
                h1[1] = emit_L1(1, Ws[1], x1)
            for b in range(NB):
                nxt = b + 2
                if nxt < NB:
                    Ws[nxt] = emit_weights(nxt)
                    h1[nxt] = emit_L1(nxt, Ws[nxt], emit_x(nxt))
                emit_L23(b, Ws.pop(b), h1.pop(b))

    nc.compile()
    _program_cache[key] = nc
    return nc


# ---------------------------------------------------------------- host
def _execute(inputs, trace=False, trace_cores=None):
    graph = np.ascontiguousarray(inputs["graph"], dtype=np.float32)
    state = np.ascontiguousarray(inputs["state"], dtype=np.float32)
    next_state = np.ascontiguousarray(inputs["next_state"], dtype=np.float32)
    W1 = np.ascontiguousarray(inputs["W1"], dtype=np.float32)
    b1 = np.ascontiguousarray(inputs["b1"], dtype=np.float32)
    W2 = np.ascontiguousarray(inputs["W2"], dtype=np.float32)
    b2 = np.ascontiguousarray(inputs["b2"], dtype=np.float32)
    W3 = np.ascontiguousarray(inputs["W3"], dtype=np.float32)
    b3 = np.ascontiguousarray(inputs["b3"], dtype=np.float32)

    B = graph.shape[0]
    NF, IN, H = W1.shape
    C = W3.shape[2]
    assert IN == graph.shape[1] + state.shape[1] + next_state.shape[1]
    assert H % 128 == 0 and C <= 128
    INP = ((IN + 127) // 128) * 128
    KO1 = INP // 128

    out_full = np.zeros((B, C), dtype=np.float32)

    # --- route: last active factor per row
    mask = graph[:, :NF] == 1.0
    active = mask.any(axis=1)
    last = (NF - 1) - np.argmax(mask[:, ::-1], axis=1)
    if not active.any():
        return (out_full, None) if trace else out_full

    rows_by_e = [np.nonzero(active & (last == e))[0] for e in range(NF)]

    plan2 = _plan_blocks([len(r) for r in rows_by_e])
    if plan2 is not None and len(plan2[0]) >= 3:
        return _execute_v2(plan2, rows_by_e, out_full, graph, state,
                           next_state, W1, b1, W2, b2, W3, b3,
                           KO1, H, C, trace, trace_cores)

    nblk = [(len(r) + BLK - 1) // BLK for r in rows_by_e]
    prof, expert_of = _make_plan(nblk)
    G, NB = len(prof), sum(prof)

    # --- pack rows into per-core block slots
    # rowmap[core] : int32 [NB, BLK], original row id or -1 (pad)
    rowmap = [np.full((NB, BLK), -1, dtype=np.int64) for _ in range(NCORES)]
    off = np.cumsum([0] + prof)  # run g occupies blocks [off[g], off[g+1])
    slots_by_e = {}
    for core in range(NCORES):
        for g in range(G):
            slots_by_e.setdefault(expert_of[core][g], []).append((core, g))
    for e in range(NF):
        rows = rows_by_e[e]
        if len(rows) == 0:
            continue
        pos = 0
        for core, g in slots_by_e.get(e, []):
            cap = prof[g] * BLK
            take = min(cap, len(rows) - pos)
            if take <= 0:
                break
            flat = rowmap[core][off[g]:off[g + 1]].reshape(-1)
            flat[:take] = rows[pos:pos + take]
            pos += take
        assert pos == len(rows), f"expert {e} rows not fully packed"

    # --- build per-core inputs
    x = np.concatenate([graph, state, next_state], axis=1)  # [B, IN]
    if INP != IN:
        x = np.concatenate([x, np.zeros((B, INP - IN), np.float32)], axis=1)
    xpad = np.concatenate([x, np.zeros((1, INP), np.float32)], axis=0)
    W1p = np.zeros((NF, INP, H), np.float32)
    W1p[:, :IN] = W1

    # Partition-major device layouts: [.., 128, KO, free] so every DMA
    # line is one contiguous 10-20KB run per partition.
    KO2 = H // 128
    W1pm = np.ascontiguousarray(
        W1p.reshape(NF, KO1, 128, H).transpose(0, 2, 1, 3).astype(NP_BF16))
    W2pm = np.ascontiguousarray(
        W2.reshape(NF, KO2, 128, H).transpose(0, 2, 1, 3).astype(NP_BF16))
    W3pm = np.ascontiguousarray(
        W3.reshape(NF, KO2, 128, C).transpose(0, 2, 1, 3).astype(NP_BF16))
    xpad = xpad.astype(NP_BF16)
    in_maps = []
    for core in range(NCORES):
        xb = xpad[rowmap[core].reshape(-1)]  # [NB*BLK, INP]; -1 -> zero row
        xb = np.ascontiguousarray(
            xb.reshape(NB, BLK, KO1, 128).transpose(0, 3, 2, 1))
        es = expert_of[core]
        in_maps.append({
            "xb": xb,
            "w1": W1pm[es],
            "w2": W2pm[es],
            "w3": W3pm[es],
            "b1": np.ascontiguousarray(b1[es]),
            "b2": np.ascontiguousarray(b2[es]),
            "b3": np.ascontiguousarray(b3[es]),
        })

    nc = _build_program(prof, KO1, KO2, H, C, BLK)
    kwargs = {}
    if trace:
        kwargs = dict(trace=True,
                      trace_cores=trace_cores or list(range(NCORES)))
    res = run_bass_kernel_spmd(nc, in_maps, list(range(NCORES)), **kwargs)

    # --- scatter back
    for core in range(NCORES):
        ob = np.asarray(res.results[core]["outb"])  # [NB, C, BLK]
        rows = ob.transpose(0, 2, 1).reshape(NB * BLK, C)
        ids = rowmap[core].reshape(-1)
        valid = ids >= 0
        out_full[ids[valid]] = rows[valid]

    return (out_full, res) if trace else out_full


def _execute_v2(plan2, rows_by_e, out_full, graph, state, next_state,
                W1, b1, W2, b2, W3, b3, KO1, H, C, trace, trace_cores):
    bsizes, expert_of, nrows_of = plan2
    NB = len(bsizes)
    NF = W1.shape[0]
    B = graph.shape[0]
    IN = W1.shape[1]
    INP = KO1 * 128
    KO2 = H // 128

    x = np.concatenate([graph, state, next_state], axis=1)
    if INP != IN:
        x = np.concatenate([x, np.zeros((B, INP - IN), np.float32)], axis=1)
    xpad = np.concatenate(
        [x, np.zeros((1, INP), np.float32)], axis=0).astype(NP_BF16)

    W1p = np.zeros((NF, INP, H), np.float32)
    W1p[:, :IN] = W1
    W1pm = np.ascontiguousarray(
        W1p.reshape(NF, KO1, 128, H).transpose(0, 2, 1, 3).astype(NP_BF16))
    W2pm = np.ascontiguousarray(
        W2.reshape(NF, KO2, 128, H).transpose(0, 2, 1, 3).astype(NP_BF16))
    W3pm = np.ascontiguousarray(
        W3.reshape(NF, KO2, 128, C).transpose(0, 2, 1, 3).astype(NP_BF16))

    # deal each expert's rows across its (core, slot) pieces in order
    ptr = [0] * NF
    in_maps = []
    rowmaps = []  # per core: list of per-block row-id arrays (-1 = pad)
    for core in range(NCORES):
        es = expert_of[core]
        rmap = []
        imap = {
            "w1": np.ascontiguousarray(W1pm[es]),
            "w2": np.ascontiguousarray(W2pm[es]),
            "w3": np.ascontiguousarray(W3pm[es]),
            "b1": np.ascontiguousarray(b1[es]),
            "b2": np.ascontiguousarray(b2[es]),
            "b3": np.ascontiguousarray(b3[es]),
        }
        in_maps.append(imap)
        rowmaps.append(rmap)
    for b in range(NB):
        bs = bsizes[b]
        for core in range(NCORES):
            e = expert_of[core][b]
            take = nrows_of[core][b]
            ids = np.full(bs, -1, dtype=np.int64)
            if take:
                ids[:take] = rows_by_e[e][ptr[e]:ptr[e] + take]
                ptr[e] += take
            rowmaps[core].append(ids)
            xb = xpad[ids]  # [bs, INP]; -1 -> zero row
            in_maps[core][f"xb{b}"] = np.ascontiguousarray(
                xb.reshape(bs, KO1, 128).transpose(2, 1, 0))
    for e in range(NF):
        assert ptr[e] == len(rows_by_e[e]), f"expert {e} rows not packed"

    nc = _build_program2(bsizes, KO1, KO2, H, C)
    kwargs = {}
    if trace:
        kwargs = dict(trace=True,
                      trace_cores=trace_cores or list(range(NCORES)))
    res = run_bass_kernel_spmd(nc, in_maps, list(range(NCORES)), **kwargs)

    for core in range(NCORES):
        for b in range(NB):
            ob = np.asarray(res.results[core][f"outb{b}"])  # [C, bs]
            ids = rowmaps[core][b]
            valid = ids >= 0
            if valid.any():
                out_full[ids[valid]] = ob.T[valid]

    return (out_full, res) if trace else out_full


def kernel(**inputs):
    return _execute(inputs)



# revision 21
# speedup vs baseline: 1.1299x; 1.1037x over previous
"""MoE-routed DIAYN discriminator kernel for 8 Trainium2 NeuronCores.

Reference semantics: x = concat([graph, state, next_state], -1); for each
row, run the 3-layer MLP of the LAST factor i<NF with graph[:, i]==1
(rows with no active factor output 0). The dense reference computes all
NF expert MLPs for every row; we instead route each row to exactly one
expert on the host, pack rows into 8 SPMD shards, and run one dense
per-expert MLP stream per core.

Sharding: rows are grouped by expert into BLK-row blocks. Every core
executes the same static "profile" of G runs (run g = prof[g] blocks);
each run uses one weight set, supplied per-core as data. A small host-side
search picks (G, prof) and an assignment of runs -> experts that covers
the actual per-expert block counts with minimal padding + weight traffic.

Device kernel (per run, per block, activations kept transposed [feat, row]):
  h1 = relu(W1^T x + b1); h2 = relu(W2^T h1 + b2); out = W3^T h2 + b3
matmuls run in bf16 (full rate on the PE, half the DMA of fp32);
PSUM accumulation, biases and the final output stay fp32.
"""

import ml_dtypes
import numpy as np

import concourse.bass as bass
import concourse.mybir as mybir
from concourse import bacc
from concourse.tile import TileContext
from concourse.bass_utils import run_bass_kernel_spmd

NCORES = 8
BLK = 272  # rows per matmul block; <=512 (PSUM bank)

F32 = mybir.dt.float32
BF16 = mybir.dt.bfloat16
NP_BF16 = ml_dtypes.bfloat16

# Rough per-core cost weights for the plan search (ns).
_COST_BLOCK = int(152 * (BLK / 2.4 + 3))  # PE ns per block (152 matmuls)
_COST_RUN = 12_000  # partially-exposed weight-set DMA per extra run

# v2 (variable-block) cost weights, from HW measurement: a bf16 matmul of
# N columns takes ~N/2.4 + 3 ns, 152 matmuls per block; a per-block
# weight-set DMA adds queue pressure and the set for block b prefetches
# during block b-2/b-1's compute.
_V2_NS_PER_ROW = 152 / 2.4
_V2_NS_PER_BLOCK = 152 * 3 + 1200
_V2_SET_DMA_NS = 13_000  # per-ring half weight set at ~200 GB/s

_program_cache = {}


# ---------------------------------------------------------------- planning
def _compositions(total, parts):
    """Non-increasing positive integer compositions of `total` into `parts`."""
    if parts == 1:
        yield (total,)
        return
    for first in range((total + parts - 1) // parts, total - parts + 2):
        for rest in _compositions(total - first, parts - 1):
            if rest[0] <= first:
                yield (first,) + rest


def _try_assign(demands, prof):
    """Greedy cover of per-expert block demands by the 8x-replicated profile.

    demands: list of (n_blocks, expert) sorted desc. Returns dict
    run_size -> list of experts (8 entries per profile slot of that size,
    padding slots filled with the largest expert) or None if infeasible.
    """
    runs = sorted([t for t in prof for _ in range(NCORES)], reverse=True)
    used = []  # (size, expert)
    for n, e in demands:
        rem = n
        while rem > 0:
            if not runs:
                return None
            # largest run <= rem, else smallest run (minimal overshoot)
            pick = None
            for i, s in enumerate(runs):
                if s <= rem:
                    pick = i
                    break
            if pick is None:
                pick = len(runs) - 1
            s = runs.pop(pick)
            used.append((s, e))
            rem -= s
    pad_expert = demands[0][1]
    for s in runs:
        used.append((s, pad_expert))
    by_size = {}
    for s, e in used:
        by_size.setdefault(s, []).append(e)
    return by_size


def _make_plan(nblk):
    """nblk: per-expert block counts. Returns (prof, expert_of[core][g])."""
    demands = sorted(
        [(n, e) for e, n in enumerate(nblk) if n > 0], reverse=True
    )
    total = sum(n for n, _ in demands)
    mincap = (total + NCORES - 1) // NCORES
    best = None
    for G in range(1, 9):
        for cap in range(mincap, mincap + 6):
            for prof in _compositions(cap, G):
                a = _try_assign(demands, prof)
                if a is None:
                    continue
                cost = cap * _COST_BLOCK + G * _COST_RUN
                if best is None or cost < best[0]:
                    best = (cost, prof, a)
    assert best is not None, "no feasible run plan found"
    _, prof, by_size = best
    queues = {s: list(es) for s, es in by_size.items()}
    expert_of = [[None] * len(prof) for _ in range(NCORES)]
    for g, s in enumerate(prof):
        for core in range(NCORES):
            expert_of[core][g] = queues[s].pop(0)
    return list(prof), expert_of


# ------------------------------------------------------------- planning v2
def _best_cover(rem, sizes, inv, limit):
    """Min-overshoot multiset of pieces covering `rem` rows.

    sizes: piece sizes desc; inv: available count per size; limit: prune
    bound on overshoot. Returns (overshoot, npieces, counts) or None.
    """
    best = [None]

    def rec(i, need, counts, used):
        if need <= 0:
            os = -need
            cand = (os, used, tuple(counts))
            if best[0] is None or cand < best[0]:
                best[0] = cand
            return
        if i == len(sizes):
            return
        if best[0] is not None and best[0][0] == 0 and used >= best[0][1]:
            return
        s = sizes[i]
        hi = min(inv[s], -(-need // s))
        for n in range(hi, -1, -1):
            if best[0] is not None and n * s - need > best[0][0] >= 0:
                continue
            counts.append(n)
            rec(i + 1, need - n * s, counts, used + n)
            counts.pop()

    rec(0, rem, [], 0)
    if best[0] is None or best[0][0] > limit:
        return None
    return best[0]


def _assign_rows(demands, bsizes):
    """Assign per-expert row demands to the 8x-replicated block slots.

    demands: [(rows, expert)] ascending. bsizes: per-core block sizes.
    Returns {expert: [(size, rows_used), ...]} or None.
    """
    sizes = sorted(set(bsizes), reverse=True)
    inv = {s: 8 * bsizes.count(s) for s in sizes}
    slack = 8 * sum(bsizes) - sum(r for r, _ in demands)
    out = {}
    for r, e in demands:
        got = _best_cover(r, sizes, inv, slack)
        if got is None:
            return None
        os, _, counts = got
        slack -= os
        pieces = []
        rem = r
        for s, n in zip(sizes, counts):
            inv[s] -= n
            for _ in range(n):
                take = min(rem, s)
                pieces.append((s, take))
                rem -= take
        out[e] = pieces
    return out


def _plan_runs(rows):
    """Pick a shared run/block structure + (core, run)->expert map.

    A run is a group of blocks sharing one weight set (one DMA per run).
    Returns (runs, expert_of[core][g], nrows_of[core][g]) or None; runs is
    a list of block-size lists, shared by all cores.
    """
    demands = sorted((r, e) for e, r in enumerate(rows) if r > 0)
    total = sum(r for r, _ in demands)
    if total == 0:
        return None

    S = list(range(512, 31, -32))
    cands = set()
    for r0 in (2, 3, 4):
        for t1 in S:
            cands.add((512 * r0, t1))
            for t2 in S:
                if t2 > t1:
                    continue
                cands.add((512 * r0, t1, t2))
                for t3 in S:
                    if t3 <= t2:
                        cands.add((512 * r0, t1, t2, t3))

    def blocks_of(caps):
        out = []
        for c in caps:
            while c > 0:
                out.append(min(512, c))
                c -= out[-1]
        return out

    def cost(caps):
        blks = blocks_of(caps)
        c = sum(caps) * _V2_NS_PER_ROW + len(blks) * _V2_NS_PER_BLOCK
        c += len(caps) * 4000  # per-run weight-set DMA pressure
        for i in range(1, len(caps)):
            c += max(0.0, _V2_SET_DMA_NS - caps[i - 1] * _V2_NS_PER_ROW)
        # sync-ring startup: W1(0) + x2 + W1(run1) (~44 us at measured
        # ring bw) must land before run 1's first L1 starts.
        j1 = -(-caps[0] // 512)
        l1d = [bs * 33.3 + 240 for bs in blks]
        l23d = [bs * 30.0 + 220 for bs in blks]
        t_run1 = sum(l1d[:j1]) + sum(l23d[:max(0, j1 - 2)])
        c += max(0.0, 44000 - t_run1)
        return c

    best = None
    for caps in sorted(cands, key=cost):
        cap8 = 8 * sum(caps)
        if not (total <= cap8 <= total + 6144):
            continue
        if best is not None and cost(caps) >= best[0]:
            continue
        assign = _assign_rows(demands, list(caps))
        if assign is None:
            assign = _assign_rows(demands[::-1], list(caps))
        if assign is None:
            continue
        best = (cost(caps), caps, assign)
    if best is None:
        return None
    _, caps, assign = best

    # queue of (expert, rows) pieces per cap; deal run-major to cores
    queues = {}
    for e, pieces in assign.items():
        for s, take in pieces:
            queues.setdefault(s, []).append((e, take))
    G = len(caps)
    expert_of = [[0] * G for _ in range(NCORES)]
    nrows_of = [[0] * G for _ in range(NCORES)]
    for g, s in enumerate(caps):
        for core in range(NCORES):
            q = queues.get(s) or []
            if q:
                e, take = q.pop(0)
            else:
                e, take = 0, 0
            expert_of[core][g] = e
            nrows_of[core][g] = take
    runs = []
    for c in caps:
        blks = []
        while c > 0:
            blks.append(min(512, c))
            c -= blks[-1]
        runs.append(blks)
    return runs, expert_of, nrows_of


# ---------------------------------------------------------------- device
def _build_program(prof, KO1, KO2, H, C, blk):
    """Build + compile the SPMD Bass program for a run profile."""
    key = (tuple(prof), KO1, KO2, H, C, blk)
    if key in _program_cache:
        return _program_cache[key]

    G = len(prof)
    NB = sum(prof)
    INP = KO1 * 128
    M1 = H // 128
    relu = mybir.ActivationFunctionType.Relu
    ident = mybir.ActivationFunctionType.Identity

    nc = bacc.Bacc("TRN2", target_bir_lowering=False, debug=False,
                   num_devices=NCORES)
    x_d = nc.dram_tensor("xb", [NB, 128, KO1, blk], BF16, kind="ExternalInput").ap()
    w1_d = nc.dram_tensor("w1", [G, 128, KO1, H], BF16, kind="ExternalInput").ap()
    w2_d = nc.dram_tensor("w2", [G, 128, KO2, H], BF16, kind="ExternalInput").ap()
    w3_d = nc.dram_tensor("w3", [G, 128, KO2, C], BF16, kind="ExternalInput").ap()
    b1_d = nc.dram_tensor("b1", [G, H], F32, kind="ExternalInput").ap()
    b2_d = nc.dram_tensor("b2", [G, H], F32, kind="ExternalInput").ap()
    b3_d = nc.dram_tensor("b3", [G, C], F32, kind="ExternalInput").ap()
    out_d = nc.dram_tensor("outb", [NB, C, blk], F32, kind="ExternalOutput").ap()

    runs = []
    for g, T in enumerate(prof):
        runs += [g] * T

    with TileContext(nc) as tc:
        with (
            tc.tile_pool(name="w", bufs=2) as wpool,
            tc.tile_pool(name="x", bufs=2) as xpool,
            tc.tile_pool(name="h1", bufs=3) as h1pool,
            tc.tile_pool(name="h2", bufs=1) as h2pool,
            tc.tile_pool(name="o", bufs=2) as opool,
            tc.tile_pool(name="ps", bufs=8, space="PSUM") as pspool,
        ):
            def emit_weights(g, x_first=None, x_hook=None, x_hook2=None):
                # Biases first (tiny, needed by the first relu). W1 as
                # per-k-tile chunks so block-0's k-outer L1 can consume
                # them as they arrive; W2 as halves (needed later).
                w1ch = []
                b1sb = b2sb = b3sb = None
                for k in range(KO1):
                    if x_first is not None:
                        nc.sync.dma_start(x_first[0][:, k, :],
                                          x_first[1][:, k, :])
                    wt = wpool.tile([128, H], BF16, tag=f"w1k{k}")
                    nc.sync.dma_start(wt[:], w1_d[g, :, k, :])
                    w1ch.append(wt)
                    if k == 0:
                        # Biases after the first chunk pair (PE can start)
                        # but well before the first relu needs them.
                        b1sb = wpool.tile([128, M1], F32, tag="b1")
                        nc.sync.dma_start(
                            b1sb[:],
                            b1_d[g].rearrange("(m p) -> p m", p=128))
                        b2sb = wpool.tile([128, M1], F32, tag="b2")
                        nc.sync.dma_start(
                            b2sb[:],
                            b2_d[g].rearrange("(m p) -> p m", p=128))
                        b3sb = wpool.tile([C, 1], F32, tag="b3")
                        nc.sync.dma_start(b3sb[:], b3_d[g][:, None])
                if x_hook is not None:
                    x_hook()
                KH2 = KO2 // 2
                w2a = wpool.tile([128, KH2, H], BF16, tag="w2a")
                nc.sync.dma_start(w2a[:], w2_d[g, :, :KH2, :])
                if x_hook2 is not None:
                    x_hook2()
                w2b = wpool.tile([128, KO2 - KH2, H], BF16, tag="w2b")
                nc.sync.dma_start(w2b[:], w2_d[g, :, KH2:, :])
                w3sb = wpool.tile([128, KO2, C], BF16, tag="w3")
                nc.sync.dma_start(w3sb[:], w3_d[g])

                def w2(k):
                    return w2a[:, k, :] if k < KH2 else w2b[:, k - KH2, :]

                return dict(w1=lambda k: w1ch[k], w2=w2, w3=w3sb,
                            b1=b1sb, b2=b2sb, b3=b3sb)

            def emit_x(b):
                # x blocks ride the second HWDGE ring (scalar), parallel
                # to the weight stream on sync.
                xsb = xpool.tile([128, KO1, blk], BF16, tag="x")
                nc.scalar.dma_start(xsb[:], x_d[b])
                return xsb

            def emit_L1(W, xsb, kouter=False):
                h1sb = h1pool.tile([128, KO2, blk], BF16, tag="h1")
                if kouter:
                    # All 8 PSUM banks accumulate in parallel; each W1
                    # chunk is fully consumed on arrival (startup mode).
                    pss = [pspool.tile([128, blk], F32, tag="ps",
                                       name=f"ps_ko{m}")
                           for m in range(M1)]
                    for k in range(KO1):
                        for m in range(M1):
                            nc.tensor.matmul(
                                pss[m][:],
                                W["w1"](k)[:, m * 128:(m + 1) * 128],
                                xsb[:, k, :],
                                start=(k == 0), stop=(k == KO1 - 1))
                    for m in range(M1):
                        nc.vector.tensor_scalar(
                            h1sb[:, m, :], pss[m][:], W["b1"][:, m:m + 1],
                            0.0, mybir.AluOpType.add, mybir.AluOpType.max)
                    return h1sb
                for m in range(M1):
                    ps = pspool.tile([128, blk], F32, tag="ps")
                    for k in range(KO1):
                        nc.tensor.matmul(
                            ps[:],
                            W["w1"](k)[:, m * 128:(m + 1) * 128],
                            xsb[:, k, :],
                            start=(k == 0), stop=(k == KO1 - 1))
                    nc.vector.tensor_scalar(
                        h1sb[:, m, :], ps[:], W["b1"][:, m:m + 1], 0.0,
                        mybir.AluOpType.add, mybir.AluOpType.max)
                return h1sb

            def emit_L23(b, W, h1sb):
                h2sb = h2pool.tile([128, KO2, blk], BF16, tag="h2")
                for m in range(M1):
                    ps = pspool.tile([128, blk], F32, tag="ps")
                    for k in range(KO2):
                        nc.tensor.matmul(
                            ps[:],
                            W["w2"](k)[:, m * 128:(m + 1) * 128],
                            h1sb[:, k, :],
                            start=(k == 0), stop=(k == KO2 - 1))
                    nc.scalar.activation(
                        h2sb[:, m, :], ps[:], relu, bias=W["b2"][:, m:m + 1])
                ps3 = pspool.tile([128, blk], F32, tag="ps")
                for k in range(KO2):
                    nc.tensor.matmul(
                        ps3[:C, :],
                        W["w3"][:, k, :],
                        h2sb[:, k, :],
                        start=(k == 0), stop=(k == KO2 - 1))
                osb = opool.tile([C, blk], F32, tag="o")
                nc.scalar.activation(
                    osb[:], ps3[:C, :], ident, bias=W["b3"][:, 0:1])
                nc.gpsimd.dma_start(out_d[b], osb[:])

            # Software pipeline, depth 2: L1 of blocks b+1/b+2 are
            # emitted before L2/L3 of block b, so weight-set DMAs and
            # ACT latency never drain the PE (esp. during the initial
            # HBM-bound weight load).
            Ws = {}
            h1 = {}

            xpre = {}

            def emit_front(b):
                g = runs[b]
                if g not in Ws:
                    Ws[g] = emit_weights(g)
                h1[b] = emit_L1(Ws[g], xpre.pop(b) if b in xpre
                                else emit_x(b))

            # Startup: x0/x1 lead the scalar ring while weights
            # stream on sync; L1(0)/L1(1) are emitted before L2(0) so
            # the PE has work during the HBM-bound weight load. Steady
            # state keeps L1 two blocks ahead of L2/L3.
            def emit_x_sync(b):
                xsb = xpool.tile([128, KO1, blk], BF16, tag="x")
                nc.sync.dma_start(xsb[:], x_d[b])
                return xsb

            # Startup: everything for the first ~3 blocks rides the sync
            # ring in consumption order (x0 interleaved with W1 chunks,
            # then x1, W2a, x2, W2b); block 0's L1 runs k-outer so each
            # W1 chunk is consumed on arrival.
            g0 = runs[0]
            if prof[0] >= 3:
                xsb0 = xpool.tile([128, KO1, blk], BF16, tag="x")
                xs = {}
                def _x12():
                    xs[1] = emit_x_sync(1)
                    xs[2] = emit_x_sync(2)

                Ws[g0] = emit_weights(g0, x_first=(xsb0, x_d[0]),
                                      x_hook=_x12)
                # x3/x4 ride the idle SWDGE ring: the scalar ring's
                # issue slot is blocked behind early L2-relus right at
                # the prologue->steady transition.
                for bb in (3, 4):
                    if bb < NB:
                        xp = xpool.tile([128, KO1, blk], BF16, tag="x",
                                        name=f"xpre{bb}")
                        nc.gpsimd.dma_start(xp[:], x_d[bb])
                        xpre[bb] = xp
                h1[0] = emit_L1(Ws[g0], xsb0, kouter=True)
                h1[1] = emit_L1(Ws[g0], xs[1])
                h1[2] = emit_L1(Ws[g0], xs[2])
                emitted = 2
            elif NB > 1 and runs[1] == g0:
                xsb0 = xpool.tile([128, KO1, blk], BF16, tag="x")
                xs1 = []
                Ws[g0] = emit_weights(g0, x_first=(xsb0, x_d[0]),
                                      x_hook=lambda: xs1.append(emit_x(1)))
                h1[0] = emit_L1(Ws[g0], xsb0, kouter=True)
                h1[1] = emit_L1(Ws[g0], xs1[0])
                emitted = 1
            else:
                emit_front(0)
                emitted = 0
            for b in range(NB):
                for nxt in range(emitted + 1, min(b + 3, NB)):
                    emit_front(nxt)
                    emitted = nxt
                if b + 4 < NB and runs[b + 4] not in Ws:
                    Ws[runs[b + 4]] = emit_weights(runs[b + 4])
                emit_L23(b, Ws[runs[b]], h1.pop(b))

    nc.compile()
    _program_cache[key] = nc
    return nc


# ---------------------------------------------------------------- device v2
def _build_program2(runs, KO1, KO2, H, C):
    """Variable-block SPMD program. Each run (group of blocks) shares one
    weight set, supplied per-core as data. Rings: W1+biases (+x1) on sync,
    W2/W3 on scalar, x blocks + outputs on gpsimd (SWDGE)."""
    key = ("v3", tuple(tuple(r) for r in runs), KO1, KO2, H, C)
    if key in _program_cache:
        return _program_cache[key]

    G = len(runs)
    blocks = [bs for r in runs for bs in r]
    run_of = [g for g, r in enumerate(runs) for _ in r]
    NB = len(blocks)
    M1 = H // 128
    KH2 = KO2 // 2
    relu = mybir.ActivationFunctionType.Relu
    ident = mybir.ActivationFunctionType.Identity

    nc = bacc.Bacc("TRN2", target_bir_lowering=False, debug=False,
                   num_devices=NCORES)
    x_ds = [nc.dram_tensor(f"xb{b}", [128, KO1, bs], BF16,
                           kind="ExternalInput").ap()
            for b, bs in enumerate(blocks)]
    out_ds = [nc.dram_tensor(f"outb{b}", [C, bs], F32,
                             kind="ExternalOutput").ap()
              for b, bs in enumerate(blocks)]
    w1_d = nc.dram_tensor("w1", [G, 128, KO1, H], BF16,
                          kind="ExternalInput").ap()
    w2_d = nc.dram_tensor("w2", [G, 128, KO2, H], BF16,
                          kind="ExternalInput").ap()
    w3_d = nc.dram_tensor("w3", [G, 128, KO2, C], BF16,
                          kind="ExternalInput").ap()
    b1_d = nc.dram_tensor("b1", [G, H], F32, kind="ExternalInput").ap()
    b2_d = nc.dram_tensor("b2", [G, H], F32, kind="ExternalInput").ap()
    b3_d = nc.dram_tensor("b3", [G, C], F32, kind="ExternalInput").ap()

    with TileContext(nc) as tc:
        with (
            tc.tile_pool(name="w", bufs=2) as wpool,
            tc.tile_pool(name="x", bufs=2) as xpool,
            tc.tile_pool(name="h1", bufs=3) as h1pool,
            tc.tile_pool(name="h2", bufs=1) as h2pool,
            tc.tile_pool(name="o", bufs=2) as opool,
            tc.tile_pool(name="ps", bufs=8, space="PSUM") as pspool,
        ):
            def emit_weights(g):
                # W1 as per-k chunks + biases on the sync ring; W2 halves
                # and W3 on the scalar ring (bufs=3 so the slot-free waits
                # are already resolved at issue time and never park the
                # scalar engine between PSUM-draining relus).
                w1ch = []
                b1sb = b2sb = b3sb = None
                for k in range(KO1):
                    wt = wpool.tile([128, H], BF16, tag=f"w1k{k}")
                    nc.sync.dma_start(wt[:], w1_d[g, :, k, :])
                    w1ch.append(wt)
                    if k == 0:
                        b1sb = wpool.tile([128, M1], F32, tag="b1", bufs=3)
                        nc.sync.dma_start(
                            b1sb[:],
                            b1_d[g].rearrange("(m p) -> p m", p=128))
                        b2sb = wpool.tile([128, M1], F32, tag="b2", bufs=3)
                        nc.sync.dma_start(
                            b2sb[:],
                            b2_d[g].rearrange("(m p) -> p m", p=128))
                        b3sb = wpool.tile([C, 1], F32, tag="b3", bufs=3)
                        nc.sync.dma_start(b3sb[:], b3_d[g][:, None])
                w2a = wpool.tile([128, KH2, H], BF16, tag="w2a", bufs=3)
                nc.scalar.dma_start(w2a[:], w2_d[g, :, :KH2, :])
                w2b = wpool.tile([128, KO2 - KH2, H], BF16, tag="w2b",
                                 bufs=3)
                nc.scalar.dma_start(w2b[:], w2_d[g, :, KH2:, :])
                w3sb = wpool.tile([128, KO2, C], BF16, tag="w3", bufs=3)
                nc.scalar.dma_start(w3sb[:], w3_d[g])

                def w2(k):
                    return w2a[:, k, :] if k < KH2 else w2b[:, k - KH2, :]

                return dict(w1=lambda k: w1ch[k], w2=w2, w3=w3sb,
                            b1=b1sb, b2=b2sb, b3=b3sb)

            def emit_x(b, ring=None):
                bs = blocks[b]
                xsb = xpool.tile([128, KO1, bs], BF16, tag="x", bufs=3)
                (ring or nc.gpsimd).dma_start(xsb[:], x_ds[b][:])
                return xsb

            def emit_L1(b, W, xsb, kouter=False):
                bs = blocks[b]
                h1sb = h1pool.tile([128, KO2, bs], BF16, tag="h1")
                if kouter:
                    # Startup: all 8 PSUM banks accumulate in parallel so
                    # each W1 chunk is fully consumed as it arrives.
                    pss = [pspool.tile([128, bs], F32, tag="ps",
                                       name=f"ps_ko{m}")
                           for m in range(M1)]
                    for k in range(KO1):
                        for m in range(M1):
                            nc.tensor.matmul(
                                pss[m][:],
                                W["w1"](k)[:, m * 128:(m + 1) * 128],
                                xsb[:, k, :],
                                start=(k == 0), stop=(k == KO1 - 1))
                    for m in range(M1):
                        nc.vector.tensor_scalar(
                            h1sb[:, m, :], pss[m][:], W["b1"][:, m:m + 1],
                            0.0, mybir.AluOpType.add, mybir.AluOpType.max)
                    return h1sb
                for m in range(M1):
                    ps = pspool.tile([128, bs], F32, tag="ps")
                    for k in range(KO1):
                        nc.tensor.matmul(
                            ps[:],
                            W["w1"](k)[:, m * 128:(m + 1) * 128],
                            xsb[:, k, :],
                            start=(k == 0), stop=(k == KO1 - 1))
                    nc.vector.tensor_scalar(
                        h1sb[:, m, :], ps[:], W["b1"][:, m:m + 1], 0.0,
                        mybir.AluOpType.add, mybir.AluOpType.max)
                return h1sb

            def emit_L23(b, W, h1sb):
                bs = blocks[b]
                h2sb = h2pool.tile([128, KO2, bs], BF16, tag="h2")
                for m in range(M1):
                    ps = pspool.tile([128, bs], F32, tag="ps")
                    for k in range(KO2):
                        nc.tensor.matmul(
                            ps[:],
                            W["w2"](k)[:, m * 128:(m + 1) * 128],
                            h1sb[:, k, :],
                            start=(k == 0), stop=(k == KO2 - 1))
                    nc.scalar.activation(
                        h2sb[:, m, :], ps[:], relu, bias=W["b2"][:, m:m + 1])
                ps3 = pspool.tile([128, bs], F32, tag="ps")
                for k in range(KO2):
                    nc.tensor.matmul(
                        ps3[:C, :],
                        W["w3"][:, k, :],
                        h2sb[:, k, :],
                        start=(k == 0), stop=(k == KO2 - 1))
                osb = opool.tile([C, bs], F32, tag="o")
                nc.scalar.activation(
                    osb[:], ps3[:C, :], ident, bias=W["b3"][:, 0:1])
                nc.gpsimd.dma_start(out_ds[b][:], osb[:])

            # Two-deep software pipeline: L1 leads L23 by two blocks and a
            # run's weight set is DMA-queued as soon as one of its blocks
            # enters the lookahead. Startup: x0 and x1 lead the scalar
            # ring (ahead of W2/W3 of run 0), W1(0) has the sync ring to
            # itself so block 0's k-outer L1 consumes chunks as they
            # arrive; x2 rides sync behind W1(0), later x on gpsimd.
            h1 = {}
            Ws = {}
            xsb0 = emit_x(0, ring=nc.scalar)
            x1 = emit_x(1, ring=nc.scalar) if NB > 1 else None
            Ws[0] = emit_weights(0)
            h1[0] = emit_L1(0, Ws[0], xsb0, kouter=True)
            if NB > 1:
                if run_of[1] not in Ws:
                    Ws[run_of[1]] = emit_weights(run_of[1])
                h1[1] = emit_L1(1, Ws[run_of[1]], x1)
            for b in range(NB):
                nxt = b + 2
                if nxt < NB:
                    g = run_of[nxt]
                    if g not in Ws:
                        Ws[g] = emit_weights(g)
                    ring = nc.sync if nxt == 2 else None
                    h1[nxt] = emit_L1(nxt, Ws[g], emit_x(nxt, ring=ring))
                emit_L23(b, Ws[run_of[b]], h1.pop(b))

    nc.compile()
    _program_cache[key] = nc
    return nc


# ---------------------------------------------------------------- host
def _execute(inputs, trace=False, trace_cores=None):
    graph = np.ascontiguousarray(inputs["graph"], dtype=np.float32)
    state = np.ascontiguousarray(inputs["state"], dtype=np.float32)
    next_state = np.ascontiguousarray(inputs["next_state"], dtype=np.float32)
    W1 = np.ascontiguousarray(inputs["W1"], dtype=np.float32)
    b1 = np.ascontiguousarray(inputs["b1"], dtype=np.float32)
    W2 = np.ascontiguousarray(inputs["W2"], dtype=np.float32)
    b2 = np.ascontiguousarray(inputs["b2"], dtype=np.float32)
    W3 = np.ascontiguousarray(inputs["W3"], dtype=np.float32)
    b3 = np.ascontiguousarray(inputs["b3"], dtype=np.float32)

    B = graph.shape[0]
    NF, IN, H = W1.shape
    C = W3.shape[2]
    assert IN == graph.shape[1] + state.shape[1] + next_state.shape[1]
    assert H % 128 == 0 and C <= 128
    INP = ((IN + 127) // 128) * 128
    KO1 = INP // 128

    out_full = np.zeros((B, C), dtype=np.float32)

    # --- route: last active factor per row
    mask = graph[:, :NF] == 1.0
    active = mask.any(axis=1)
    last = (NF - 1) - np.argmax(mask[:, ::-1], axis=1)
    if not active.any():
        return (out_full, None) if trace else out_full

    rows_by_e = [np.nonzero(active & (last == e))[0] for e in range(NF)]

    plan2 = _plan_runs([len(r) for r in rows_by_e])
    if plan2 is not None and sum(len(r) for r in plan2[0]) >= 3:
        return _execute_v2(plan2, rows_by_e, out_full, graph, state,
                           next_state, W1, b1, W2, b2, W3, b3,
                           KO1, H, C, trace, trace_cores)

    nblk = [(len(r) + BLK - 1) // BLK for r in rows_by_e]
    prof, expert_of = _make_plan(nblk)
    G, NB = len(prof), sum(prof)

    # --- pack rows into per-core block slots
    # rowmap[core] : int32 [NB, BLK], original row id or -1 (pad)
    rowmap = [np.full((NB, BLK), -1, dtype=np.int64) for _ in range(NCORES)]
    off = np.cumsum([0] + prof)  # run g occupies blocks [off[g], off[g+1])
    slots_by_e = {}
    for core in range(NCORES):
        for g in range(G):
            slots_by_e.setdefault(expert_of[core][g], []).append((core, g))
    for e in range(NF):
        rows = rows_by_e[e]
        if len(rows) == 0:
            continue
        pos = 0
        for core, g in slots_by_e.get(e, []):
            cap = prof[g] * BLK
            take = min(cap, len(rows) - pos)
            if take <= 0:
                break
            flat = rowmap[core][off[g]:off[g + 1]].reshape(-1)
            flat[:take] = rows[pos:pos + take]
            pos += take
        assert pos == len(rows), f"expert {e} rows not fully packed"

    # --- build per-core inputs
    x = np.concatenate([graph, state, next_state], axis=1)  # [B, IN]
    if INP != IN:
        x = np.concatenate([x, np.zeros((B, INP - IN), np.float32)], axis=1)
    xpad = np.concatenate([x, np.zeros((1, INP), np.float32)], axis=0)
    W1p = np.zeros((NF, INP, H), np.float32)
    W1p[:, :IN] = W1

    # Partition-major device layouts: [.., 128, KO, free] so every DMA
    # line is one contiguous 10-20KB run per partition.
    KO2 = H // 128
    W1pm = np.ascontiguousarray(
        W1p.reshape(NF, KO1, 128, H).transpose(0, 2, 1, 3).astype(NP_BF16))
    W2pm = np.ascontiguousarray(
        W2.reshape(NF, KO2, 128, H).transpose(0, 2, 1, 3).astype(NP_BF16))
    W3pm = np.ascontiguousarray(
        W3.reshape(NF, KO2, 128, C).transpose(0, 2, 1, 3).astype(NP_BF16))
    xpad = xpad.astype(NP_BF16)
    in_maps = []
    for core in range(NCORES):
        xb = xpad[rowmap[core].reshape(-1)]  # [NB*BLK, INP]; -1 -> zero row
        xb = np.ascontiguousarray(
            xb.reshape(NB, BLK, KO1, 128).transpose(0, 3, 2, 1))
        es = expert_of[core]
        in_maps.append({
            "xb": xb,
            "w1": W1pm[es],
            "w2": W2pm[es],
            "w3": W3pm[es],
            "b1": np.ascontiguousarray(b1[es]),
            "b2": np.ascontiguousarray(b2[es]),
            "b3": np.ascontiguousarray(b3[es]),
        })

    nc = _build_program(prof, KO1, KO2, H, C, BLK)
    kwargs = {}
    if trace:
        kwargs = dict(trace=True,
                      trace_cores=trace_cores or list(range(NCORES)))
    res = run_bass_kernel_spmd(nc, in_maps, list(range(NCORES)), **kwargs)

    # --- scatter back
    for core in range(NCORES):
        ob = np.asarray(res.results[core]["outb"])  # [NB, C, BLK]
        rows = ob.transpose(0, 2, 1).reshape(NB * BLK, C)
        ids = rowmap[core].reshape(-1)
        valid = ids >= 0
        out_full[ids[valid]] = rows[valid]

    return (out_full, res) if trace else out_full


def _execute_v2(plan2, rows_by_e, out_full, graph, state, next_state,
                W1, b1, W2, b2, W3, b3, KO1, H, C, trace, trace_cores):
    runs, expert_of, nrows_of = plan2
    G = len(runs)
    bsizes = [bs for r in runs for bs in r]
    run_of = [g for g, r in enumerate(runs) for _ in r]
    NB = len(bsizes)
    NF = W1.shape[0]
    B = graph.shape[0]
    IN = W1.shape[1]
    INP = KO1 * 128
    KO2 = H // 128

    x = np.concatenate([graph, state, next_state], axis=1)
    if INP != IN:
        x = np.concatenate([x, np.zeros((B, INP - IN), np.float32)], axis=1)
    xpad = np.concatenate(
        [x, np.zeros((1, INP), np.float32)], axis=0).astype(NP_BF16)

    W1p = np.zeros((NF, INP, H), np.float32)
    W1p[:, :IN] = W1
    W1pm = np.ascontiguousarray(
        W1p.reshape(NF, KO1, 128, H).transpose(0, 2, 1, 3).astype(NP_BF16))
    W2pm = np.ascontiguousarray(
        W2.reshape(NF, KO2, 128, H).transpose(0, 2, 1, 3).astype(NP_BF16))
    W3pm = np.ascontiguousarray(
        W3.reshape(NF, KO2, 128, C).transpose(0, 2, 1, 3).astype(NP_BF16))

    # deal each expert's rows across its (core, run) pieces in order
    ptr = [0] * NF
    in_maps = []
    rowmaps = []  # per core: list of per-block row-id arrays (-1 = pad)
    for core in range(NCORES):
        es = expert_of[core]  # one expert per run
        imap = {
            "w1": np.ascontiguousarray(W1pm[es]),
            "w2": np.ascontiguousarray(W2pm[es]),
            "w3": np.ascontiguousarray(W3pm[es]),
            "b1": np.ascontiguousarray(b1[es]),
            "b2": np.ascontiguousarray(b2[es]),
            "b3": np.ascontiguousarray(b3[es]),
        }
        rmap = []
        for g in range(G):
            e = es[g]
            take = nrows_of[core][g]
            ids_run = np.full(sum(runs[g]), -1, dtype=np.int64)
            if take:
                ids_run[:take] = rows_by_e[e][ptr[e]:ptr[e] + take]
                ptr[e] += take
            off = 0
            for bs in runs[g]:
                rmap.append(ids_run[off:off + bs])
                off += bs
        for b in range(NB):
            xb = xpad[rmap[b]]  # [bs, INP]; -1 -> zero row
            imap[f"xb{b}"] = np.ascontiguousarray(
                xb.reshape(bsizes[b], KO1, 128).transpose(2, 1, 0))
        in_maps.append(imap)
        rowmaps.append(rmap)
    for e in range(NF):
        assert ptr[e] == len(rows_by_e[e]), f"expert {e} rows not packed"

    nc = _build_program2(runs, KO1, KO2, H, C)
    kwargs = {}
    if trace:
        kwargs = dict(trace=True,
                      trace_cores=trace_cores or list(range(NCORES)))
    res = run_bass_kernel_spmd(nc, in_maps, list(range(NCORES)), **kwargs)

    for core in range(NCORES):
        for b in range(NB):
            ob = np.asarray(res.results[core][f"outb{b}"])  # [C, bs]
            ids = rowmaps[core][b]
            valid = ids >= 0
            if valid.any():
                out_full[ids[valid]] = ob.T[valid]

    return (out_full, res) if trace else out_full


def kernel(**inputs):
    return _execute(inputs)



# revision 25
# speedup vs baseline: 1.1453x; 1.0136x over previous
"""MoE-routed DIAYN discriminator kernel for 8 Trainium2 NeuronCores.

Reference semantics: x = concat([graph, state, next_state], -1); for each
row, run the 3-layer MLP of the LAST factor i<NF with graph[:, i]==1
(rows with no active factor output 0). The dense reference computes all
NF expert MLPs for every row; we instead route each row to exactly one
expert on the host, pack rows into 8 SPMD shards, and run one dense
per-expert MLP stream per core.

Sharding: rows are grouped by expert into BLK-row blocks. Every core
executes the same static "profile" of G runs (run g = prof[g] blocks);
each run uses one weight set, supplied per-core as data. A small host-side
search picks (G, prof) and an assignment of runs -> experts that covers
the actual per-expert block counts with minimal padding + weight traffic.

Device kernel (per run, per block, activations kept transposed [feat, row]):
  h1 = relu(W1^T x + b1); h2 = relu(W2^T h1 + b2); out = W3^T h2 + b3
matmuls run in bf16 (full rate on the PE, half the DMA of fp32);
PSUM accumulation, biases and the final output stay fp32.
"""

import ml_dtypes
import numpy as np

import concourse.bass as bass
import concourse.mybir as mybir
from concourse import bacc
from concourse.tile import TileContext
from concourse.bass_utils import run_bass_kernel_spmd

NCORES = 8
BLK = 272  # rows per matmul block; <=512 (PSUM bank)

F32 = mybir.dt.float32
BF16 = mybir.dt.bfloat16
NP_BF16 = ml_dtypes.bfloat16

# Rough per-core cost weights for the plan search (ns).
_COST_BLOCK = int(152 * (BLK / 2.4 + 3))  # PE ns per block (152 matmuls)
_COST_RUN = 12_000  # partially-exposed weight-set DMA per extra run

# v2 (variable-block) cost weights, from HW measurement: a bf16 matmul of
# N columns takes ~N/2.4 + 3 ns, 152 matmuls per block; a per-block
# weight-set DMA adds queue pressure and the set for block b prefetches
# during block b-2/b-1's compute.
_V2_NS_PER_ROW = 152 / 2.4
_V2_NS_PER_BLOCK = 152 * 3 + 1200
_V2_SET_DMA_NS = 13_000  # per-ring half weight set at ~200 GB/s

_program_cache = {}


# ---------------------------------------------------------------- planning
def _compositions(total, parts):
    """Non-increasing positive integer compositions of `total` into `parts`."""
    if parts == 1:
        yield (total,)
        return
    for first in range((total + parts - 1) // parts, total - parts + 2):
        for rest in _compositions(total - first, parts - 1):
            if rest[0] <= first:
                yield (first,) + rest


def _try_assign(demands, prof):
    """Greedy cover of per-expert block demands by the 8x-replicated profile.

    demands: list of (n_blocks, expert) sorted desc. Returns dict
    run_size -> list of experts (8 entries per profile slot of that size,
    padding slots filled with the largest expert) or None if infeasible.
    """
    runs = sorted([t for t in prof for _ in range(NCORES)], reverse=True)
    used = []  # (size, expert)
    for n, e in demands:
        rem = n
        while rem > 0:
            if not runs:
                return None
            # largest run <= rem, else smallest run (minimal overshoot)
            pick = None
            for i, s in enumerate(runs):
                if s <= rem:
                    pick = i
                    break
            if pick is None:
                pick = len(runs) - 1
            s = runs.pop(pick)
            used.append((s, e))
            rem -= s
    pad_expert = demands[0][1]
    for s in runs:
        used.append((s, pad_expert))
    by_size = {}
    for s, e in used:
        by_size.setdefault(s, []).append(e)
    return by_size


def _make_plan(nblk):
    """nblk: per-expert block counts. Returns (prof, expert_of[core][g])."""
    demands = sorted(
        [(n, e) for e, n in enumerate(nblk) if n > 0], reverse=True
    )
    total = sum(n for n, _ in demands)
    mincap = (total + NCORES - 1) // NCORES
    best = None
    for G in range(1, 9):
        for cap in range(mincap, mincap + 6):
            for prof in _compositions(cap, G):
                a = _try_assign(demands, prof)
                if a is None:
                    continue
                cost = cap * _COST_BLOCK + G * _COST_RUN
                if best is None or cost < best[0]:
                    best = (cost, prof, a)
    assert best is not None, "no feasible run plan found"
    _, prof, by_size = best
    queues = {s: list(es) for s, es in by_size.items()}
    expert_of = [[None] * len(prof) for _ in range(NCORES)]
    for g, s in enumerate(prof):
        for core in range(NCORES):
            expert_of[core][g] = queues[s].pop(0)
    return list(prof), expert_of


# ------------------------------------------------------------- planning v2
def _best_cover(rem, sizes, inv, limit):
    """Min-overshoot multiset of pieces covering `rem` rows.

    sizes: piece sizes desc; inv: available count per size; limit: prune
    bound on overshoot. Returns (overshoot, npieces, counts) or None.
    """
    best = [None]

    def rec(i, need, counts, used):
        if need <= 0:
            os = -need
            cand = (os, used, tuple(counts))
            if best[0] is None or cand < best[0]:
                best[0] = cand
            return
        if i == len(sizes):
            return
        if best[0] is not None and best[0][0] == 0 and used >= best[0][1]:
            return
        s = sizes[i]
        hi = min(inv[s], -(-need // s))
        for n in range(hi, -1, -1):
            if best[0] is not None and n * s - need > best[0][0] >= 0:
                continue
            counts.append(n)
            rec(i + 1, need - n * s, counts, used + n)
            counts.pop()

    rec(0, rem, [], 0)
    if best[0] is None or best[0][0] > limit:
        return None
    return best[0]


def _assign_rows(demands, bsizes):
    """Assign per-expert row demands to the 8x-replicated block slots.

    demands: [(rows, expert)] ascending. bsizes: per-core block sizes.
    Returns {expert: [(size, rows_used), ...]} or None.
    """
    sizes = sorted(set(bsizes), reverse=True)
    inv = {s: 8 * bsizes.count(s) for s in sizes}
    slack = 8 * sum(bsizes) - sum(r for r, _ in demands)
    out = {}
    for r, e in demands:
        got = _best_cover(r, sizes, inv, slack)
        if got is None:
            return None
        os, _, counts = got
        slack -= os
        pieces = []
        rem = r
        for s, n in zip(sizes, counts):
            inv[s] -= n
            for _ in range(n):
                take = min(rem, s)
                pieces.append((s, take))
                rem -= take
        out[e] = pieces
    return out


def _plan_runs(rows):
    """Pick a shared run/block structure + (core, run)->expert map.

    A run is a group of blocks sharing one weight set (one DMA per run).
    Returns (runs, expert_of[core][g], nrows_of[core][g]) or None; runs is
    a list of block-size lists, shared by all cores.
    """
    demands = sorted((r, e) for e, r in enumerate(rows) if r > 0)
    total = sum(r for r, _ in demands)
    if total == 0:
        return None

    S = list(range(512, 31, -32))
    cands = set()
    for r0 in (2, 3, 4):
        for t1 in S:
            cands.add((512 * r0, t1))
            for t2 in S:
                if t2 > t1:
                    continue
                cands.add((512 * r0, t1, t2))
                for t3 in S:
                    if t3 <= t2:
                        cands.add((512 * r0, t1, t2, t3))

    def blocks_of(caps):
        out = []
        for c in caps:
            while c > 0:
                out.append(min(512, c))
                c -= out[-1]
        return out

    def cost(caps):
        blks = blocks_of(caps)
        c = sum(caps) * _V2_NS_PER_ROW + len(blks) * _V2_NS_PER_BLOCK
        c += len(caps) * 4000  # per-run weight-set DMA pressure
        for i in range(1, len(caps)):
            c += max(0.0, _V2_SET_DMA_NS - caps[i - 1] * _V2_NS_PER_ROW)
        # sync-ring startup: W1(0) + x2 + W1(run1) (~44 us at measured
        # ring bw) must land before run 1's first L1 starts.
        j1 = -(-caps[0] // 512)
        l1d = [bs * 33.3 + 240 for bs in blks]
        l23d = [bs * 30.0 + 220 for bs in blks]
        t_run1 = sum(l1d[:j1]) + sum(l23d[:max(0, j1 - 2)])
        c += max(0.0, 44000 - t_run1)
        return c

    best = None
    for caps in sorted(cands, key=cost):
        cap8 = 8 * sum(caps)
        if not (total <= cap8 <= total + 6144):
            continue
        if best is not None and cost(caps) >= best[0]:
            continue
        assign = _assign_rows(demands, list(caps))
        if assign is None:
            assign = _assign_rows(demands[::-1], list(caps))
        if assign is None:
            continue
        best = (cost(caps), caps, assign)
    if best is None:
        return None
    _, caps, assign = best

    # queue of (expert, rows) pieces per cap; deal run-major to cores
    queues = {}
    for e, pieces in assign.items():
        for s, take in pieces:
            queues.setdefault(s, []).append((e, take))
    G = len(caps)
    expert_of = [[0] * G for _ in range(NCORES)]
    nrows_of = [[0] * G for _ in range(NCORES)]
    for g, s in enumerate(caps):
        for core in range(NCORES):
            q = queues.get(s) or []
            if q:
                e, take = q.pop(0)
            else:
                e, take = 0, 0
            expert_of[core][g] = e
            nrows_of[core][g] = take
    runs = []
    for c in caps:
        blks = []
        while c > 0:
            blks.append(min(512, c))
            c -= blks[-1]
        runs.append(blks)
    return runs, expert_of, nrows_of


# ---------------------------------------------------------------- device
def _build_program(prof, KO1, KO2, H, C, blk):
    """Build + compile the SPMD Bass program for a run profile."""
    key = (tuple(prof), KO1, KO2, H, C, blk)
    if key in _program_cache:
        return _program_cache[key]

    G = len(prof)
    NB = sum(prof)
    INP = KO1 * 128
    M1 = H // 128
    relu = mybir.ActivationFunctionType.Relu
    ident = mybir.ActivationFunctionType.Identity

    nc = bacc.Bacc("TRN2", target_bir_lowering=False, debug=False,
                   num_devices=NCORES)
    x_d = nc.dram_tensor("xb", [NB, 128, KO1, blk], BF16, kind="ExternalInput").ap()
    w1_d = nc.dram_tensor("w1", [G, 128, KO1, H], BF16, kind="ExternalInput").ap()
    w2_d = nc.dram_tensor("w2", [G, 128, KO2, H], BF16, kind="ExternalInput").ap()
    w3_d = nc.dram_tensor("w3", [G, 128, KO2, C], BF16, kind="ExternalInput").ap()
    b1_d = nc.dram_tensor("b1", [G, H], F32, kind="ExternalInput").ap()
    b2_d = nc.dram_tensor("b2", [G, H], F32, kind="ExternalInput").ap()
    b3_d = nc.dram_tensor("b3", [G, C], F32, kind="ExternalInput").ap()
    out_d = nc.dram_tensor("outb", [NB, C, blk], F32, kind="ExternalOutput").ap()

    runs = []
    for g, T in enumerate(prof):
        runs += [g] * T

    with TileContext(nc) as tc:
        with (
            tc.tile_pool(name="w", bufs=2) as wpool,
            tc.tile_pool(name="x", bufs=2) as xpool,
            tc.tile_pool(name="h1", bufs=3) as h1pool,
            tc.tile_pool(name="h2", bufs=1) as h2pool,
            tc.tile_pool(name="o", bufs=2) as opool,
            tc.tile_pool(name="ps", bufs=8, space="PSUM") as pspool,
        ):
            def emit_weights(g, x_first=None, x_hook=None, x_hook2=None):
                # Biases first (tiny, needed by the first relu). W1 as
                # per-k-tile chunks so block-0's k-outer L1 can consume
                # them as they arrive; W2 as halves (needed later).
                w1ch = []
                b1sb = b2sb = b3sb = None
                for k in range(KO1):
                    if x_first is not None:
                        nc.sync.dma_start(x_first[0][:, k, :],
                                          x_first[1][:, k, :])
                    wt = wpool.tile([128, H], BF16, tag=f"w1k{k}")
                    nc.sync.dma_start(wt[:], w1_d[g, :, k, :])
                    w1ch.append(wt)
                    if k == 0:
                        # Biases after the first chunk pair (PE can start)
                        # but well before the first relu needs them.
                        b1sb = wpool.tile([128, M1], F32, tag="b1")
                        nc.sync.dma_start(
                            b1sb[:],
                            b1_d[g].rearrange("(m p) -> p m", p=128))
                        b2sb = wpool.tile([128, M1], F32, tag="b2")
                        nc.sync.dma_start(
                            b2sb[:],
                            b2_d[g].rearrange("(m p) -> p m", p=128))
                        b3sb = wpool.tile([C, 1], F32, tag="b3")
                        nc.sync.dma_start(b3sb[:], b3_d[g][:, None])
                if x_hook is not None:
                    x_hook()
                KH2 = KO2 // 2
                w2a = wpool.tile([128, KH2, H], BF16, tag="w2a")
                nc.sync.dma_start(w2a[:], w2_d[g, :, :KH2, :])
                if x_hook2 is not None:
                    x_hook2()
                w2b = wpool.tile([128, KO2 - KH2, H], BF16, tag="w2b")
                nc.sync.dma_start(w2b[:], w2_d[g, :, KH2:, :])
                w3sb = wpool.tile([128, KO2, C], BF16, tag="w3")
                nc.sync.dma_start(w3sb[:], w3_d[g])

                def w2(k):
                    return w2a[:, k, :] if k < KH2 else w2b[:, k - KH2, :]

                return dict(w1=lambda k: w1ch[k], w2=w2, w3=w3sb,
                            b1=b1sb, b2=b2sb, b3=b3sb)

            def emit_x(b):
                # x blocks ride the second HWDGE ring (scalar), parallel
                # to the weight stream on sync.
                xsb = xpool.tile([128, KO1, blk], BF16, tag="x")
                nc.scalar.dma_start(xsb[:], x_d[b])
                return xsb

            def emit_L1(W, xsb, kouter=False):
                h1sb = h1pool.tile([128, KO2, blk], BF16, tag="h1")
                if kouter:
                    # All 8 PSUM banks accumulate in parallel; each W1
                    # chunk is fully consumed on arrival (startup mode).
                    pss = [pspool.tile([128, blk], F32, tag="ps",
                                       name=f"ps_ko{m}")
                           for m in range(M1)]
                    for k in range(KO1):
                        for m in range(M1):
                            nc.tensor.matmul(
                                pss[m][:],
                                W["w1"](k)[:, m * 128:(m + 1) * 128],
                                xsb[:, k, :],
                                start=(k == 0), stop=(k == KO1 - 1))
                    for m in range(M1):
                        nc.vector.tensor_scalar(
                            h1sb[:, m, :], pss[m][:], W["b1"][:, m:m + 1],
                            0.0, mybir.AluOpType.add, mybir.AluOpType.max)
                    return h1sb
                for m in range(M1):
                    ps = pspool.tile([128, blk], F32, tag="ps")
                    for k in range(KO1):
                        nc.tensor.matmul(
                            ps[:],
                            W["w1"](k)[:, m * 128:(m + 1) * 128],
                            xsb[:, k, :],
                            start=(k == 0), stop=(k == KO1 - 1))
                    nc.vector.tensor_scalar(
                        h1sb[:, m, :], ps[:], W["b1"][:, m:m + 1], 0.0,
                        mybir.AluOpType.add, mybir.AluOpType.max)
                return h1sb

            def emit_L23(b, W, h1sb):
                h2sb = h2pool.tile([128, KO2, blk], BF16, tag="h2")
                for m in range(M1):
                    ps = pspool.tile([128, blk], F32, tag="ps")
                    for k in range(KO2):
                        nc.tensor.matmul(
                            ps[:],
                            W["w2"](k)[:, m * 128:(m + 1) * 128],
                            h1sb[:, k, :],
                            start=(k == 0), stop=(k == KO2 - 1))
                    nc.scalar.activation(
                        h2sb[:, m, :], ps[:], relu, bias=W["b2"][:, m:m + 1])
                ps3 = pspool.tile([128, blk], F32, tag="ps")
                for k in range(KO2):
                    nc.tensor.matmul(
                        ps3[:C, :],
                        W["w3"][:, k, :],
                        h2sb[:, k, :],
                        start=(k == 0), stop=(k == KO2 - 1))
                osb = opool.tile([C, blk], F32, tag="o")
                nc.scalar.activation(
                    osb[:], ps3[:C, :], ident, bias=W["b3"][:, 0:1])
                nc.gpsimd.dma_start(out_d[b], osb[:])

            # Software pipeline, depth 2: L1 of blocks b+1/b+2 are
            # emitted before L2/L3 of block b, so weight-set DMAs and
            # ACT latency never drain the PE (esp. during the initial
            # HBM-bound weight load).
            Ws = {}
            h1 = {}

            xpre = {}

            def emit_front(b):
                g = runs[b]
                if g not in Ws:
                    Ws[g] = emit_weights(g)
                h1[b] = emit_L1(Ws[g], xpre.pop(b) if b in xpre
                                else emit_x(b))

            # Startup: x0/x1 lead the scalar ring while weights
            # stream on sync; L1(0)/L1(1) are emitted before L2(0) so
            # the PE has work during the HBM-bound weight load. Steady
            # state keeps L1 two blocks ahead of L2/L3.
            def emit_x_sync(b):
                xsb = xpool.tile([128, KO1, blk], BF16, tag="x")
                nc.sync.dma_start(xsb[:], x_d[b])
                return xsb

            # Startup: everything for the first ~3 blocks rides the sync
            # ring in consumption order (x0 interleaved with W1 chunks,
            # then x1, W2a, x2, W2b); block 0's L1 runs k-outer so each
            # W1 chunk is consumed on arrival.
            g0 = runs[0]
            if prof[0] >= 3:
                xsb0 = xpool.tile([128, KO1, blk], BF16, tag="x")
                xs = {}
                def _x12():
                    xs[1] = emit_x_sync(1)
                    xs[2] = emit_x_sync(2)

                Ws[g0] = emit_weights(g0, x_first=(xsb0, x_d[0]),
                                      x_hook=_x12)
                # x3/x4 ride the idle SWDGE ring: the scalar ring's
                # issue slot is blocked behind early L2-relus right at
                # the prologue->steady transition.
                for bb in (3, 4):
                    if bb < NB:
                        xp = xpool.tile([128, KO1, blk], BF16, tag="x",
                                        name=f"xpre{bb}")
                        nc.gpsimd.dma_start(xp[:], x_d[bb])
                        xpre[bb] = xp
                h1[0] = emit_L1(Ws[g0], xsb0, kouter=True)
                h1[1] = emit_L1(Ws[g0], xs[1])
                h1[2] = emit_L1(Ws[g0], xs[2])
                emitted = 2
            elif NB > 1 and runs[1] == g0:
                xsb0 = xpool.tile([128, KO1, blk], BF16, tag="x")
                xs1 = []
                Ws[g0] = emit_weights(g0, x_first=(xsb0, x_d[0]),
                                      x_hook=lambda: xs1.append(emit_x(1)))
                h1[0] = emit_L1(Ws[g0], xsb0, kouter=True)
                h1[1] = emit_L1(Ws[g0], xs1[0])
                emitted = 1
            else:
                emit_front(0)
                emitted = 0
            for b in range(NB):
                for nxt in range(emitted + 1, min(b + 3, NB)):
                    emit_front(nxt)
                    emitted = nxt
                if b + 4 < NB and runs[b + 4] not in Ws:
                    Ws[runs[b + 4]] = emit_weights(runs[b + 4])
                emit_L23(b, Ws[runs[b]], h1.pop(b))

    nc.compile()
    _program_cache[key] = nc
    return nc


# ---------------------------------------------------------------- device v2
def _build_program2(runs, KO1, KO2, H, C):
    """Variable-block SPMD program. Each run (group of blocks) shares one
    weight set, supplied per-core as data. Rings: W1+biases (+x1) on sync,
    W2/W3 on scalar, x blocks + outputs on gpsimd (SWDGE)."""
    key = ("v3", tuple(tuple(r) for r in runs), KO1, KO2, H, C)
    if key in _program_cache:
        return _program_cache[key]

    G = len(runs)
    blocks = [bs for r in runs for bs in r]
    run_of = [g for g, r in enumerate(runs) for _ in r]
    NB = len(blocks)
    M1 = H // 128
    KH2 = KO2 // 2
    relu = mybir.ActivationFunctionType.Relu
    ident = mybir.ActivationFunctionType.Identity

    nc = bacc.Bacc("TRN2", target_bir_lowering=False, debug=False,
                   num_devices=NCORES)
    x_ds = [nc.dram_tensor(f"xb{b}", [128, KO1, bs], BF16,
                           kind="ExternalInput").ap()
            for b, bs in enumerate(blocks)]
    out_ds = [nc.dram_tensor(f"outb{b}", [C, bs], F32,
                             kind="ExternalOutput").ap()
              for b, bs in enumerate(blocks)]
    w1_d = nc.dram_tensor("w1", [G, 128, KO1, H], BF16,
                          kind="ExternalInput").ap()
    w2_d = nc.dram_tensor("w2", [G, 128, KO2, H], BF16,
                          kind="ExternalInput").ap()
    w3_d = nc.dram_tensor("w3", [G, 128, KO2, C], BF16,
                          kind="ExternalInput").ap()
    # biases pre-packed partition-major on the host: cols [0,M1) = b1,
    # [M1,2*M1) = b2, col 2*M1 = b3 (first C partitions). One contiguous
    # DMA per set -- tiny strided bias transfers wedge the sync queue for
    # ~6 us each and starve the startup W1 stream.
    ball_d = nc.dram_tensor("ball", [G, 128, 2 * M1 + 1], F32,
                            kind="ExternalInput").ap()

    with TileContext(nc) as tc:
        with (
            tc.tile_pool(name="w", bufs=2) as wpool,
            tc.tile_pool(name="x", bufs=2) as xpool,
            tc.tile_pool(name="h1", bufs=3) as h1pool,
            tc.tile_pool(name="h2", bufs=1) as h2pool,
            tc.tile_pool(name="o", bufs=2) as opool,
            tc.tile_pool(name="ps", bufs=8, space="PSUM") as pspool,
        ):
            def emit_weights(g):
                # W1 as per-k chunks + biases on the sync ring; W2 halves
                # and W3 on the scalar ring (bufs=3 so the slot-free waits
                # are already resolved at issue time and never park the
                # scalar engine between PSUM-draining relus).
                w1ch = []
                ball = None
                for k in range(KO1):
                    wt = wpool.tile([128, H], BF16, tag=f"w1k{k}")
                    nc.sync.dma_start(wt[:], w1_d[g, :, k, :])
                    w1ch.append(wt)
                    if k == 0:
                        ball = wpool.tile([128, 2 * M1 + 1], F32,
                                          tag="ball", bufs=3)
                        nc.sync.dma_start(ball[:], ball_d[g])
                w2a = wpool.tile([128, KH2, H], BF16, tag="w2a", bufs=3)
                nc.scalar.dma_start(w2a[:], w2_d[g, :, :KH2, :])
                w2b = wpool.tile([128, KO2 - KH2, H], BF16, tag="w2b",
                                 bufs=3)
                nc.scalar.dma_start(w2b[:], w2_d[g, :, KH2:, :])
                w3sb = wpool.tile([128, KO2, C], BF16, tag="w3", bufs=3)
                nc.scalar.dma_start(w3sb[:], w3_d[g])

                def w2(k):
                    return w2a[:, k, :] if k < KH2 else w2b[:, k - KH2, :]

                return dict(w1=lambda k: w1ch[k], w2=w2, w3=w3sb,
                            b1=ball[:, 0:M1], b2=ball[:, M1:2 * M1],
                            b3=ball[:C, 2 * M1:2 * M1 + 1])

            def emit_x(b, ring=None):
                bs = blocks[b]
                xsb = xpool.tile([128, KO1, bs], BF16, tag="x", bufs=3)
                (ring or nc.gpsimd).dma_start(xsb[:], x_ds[b][:])
                return xsb

            def emit_L1(b, W, xsb, kouter=False):
                bs = blocks[b]
                h1sb = h1pool.tile([128, KO2, bs], BF16, tag="h1")
                if kouter:
                    # Startup: all 8 PSUM banks accumulate in parallel so
                    # each W1 chunk is fully consumed as it arrives.
                    pss = [pspool.tile([128, bs], F32, tag="ps",
                                       name=f"ps_ko{m}")
                           for m in range(M1)]
                    for k in range(KO1):
                        for m in range(M1):
                            nc.tensor.matmul(
                                pss[m][:],
                                W["w1"](k)[:, m * 128:(m + 1) * 128],
                                xsb[:, k, :],
                                start=(k == 0), stop=(k == KO1 - 1))
                    for m in range(M1):
                        nc.vector.tensor_scalar(
                            h1sb[:, m, :], pss[m][:], W["b1"][:, m:m + 1],
                            0.0, mybir.AluOpType.add, mybir.AluOpType.max)
                    return h1sb
                for m in range(M1):
                    ps = pspool.tile([128, bs], F32, tag="ps")
                    for k in range(KO1):
                        nc.tensor.matmul(
                            ps[:],
                            W["w1"](k)[:, m * 128:(m + 1) * 128],
                            xsb[:, k, :],
                            start=(k == 0), stop=(k == KO1 - 1))
                    nc.vector.tensor_scalar(
                        h1sb[:, m, :], ps[:], W["b1"][:, m:m + 1], 0.0,
                        mybir.AluOpType.add, mybir.AluOpType.max)
                return h1sb

            def emit_L23(b, W, h1sb):
                bs = blocks[b]
                h2sb = h2pool.tile([128, KO2, bs], BF16, tag="h2")
                for m in range(M1):
                    ps = pspool.tile([128, bs], F32, tag="ps")
                    for k in range(KO2):
                        nc.tensor.matmul(
                            ps[:],
                            W["w2"](k)[:, m * 128:(m + 1) * 128],
                            h1sb[:, k, :],
                            start=(k == 0), stop=(k == KO2 - 1))
                    nc.scalar.activation(
                        h2sb[:, m, :], ps[:], relu, bias=W["b2"][:, m:m + 1])
                ps3 = pspool.tile([128, bs], F32, tag="ps")
                for k in range(KO2):
                    nc.tensor.matmul(
                        ps3[:C, :],
                        W["w3"][:, k, :],
                        h2sb[:, k, :],
                        start=(k == 0), stop=(k == KO2 - 1))
                osb = opool.tile([C, bs], F32, tag="o")
                nc.scalar.activation(
                    osb[:], ps3[:C, :], ident, bias=W["b3"][:, 0:1])
                nc.gpsimd.dma_start(out_ds[b][:], osb[:])

            # Two-deep software pipeline: L1 leads L23 by two blocks and a
            # run's weight set is DMA-queued as soon as one of its blocks
            # enters the lookahead. Startup: x0 and x1 lead the scalar
            # ring (ahead of W2/W3 of run 0), W1(0) has the sync ring to
            # itself so block 0's k-outer L1 consumes chunks as they
            # arrive; x2 rides sync behind W1(0), later x on gpsimd.
            h1 = {}
            Ws = {}
            xsb0 = emit_x(0, ring=nc.scalar)
            x1 = emit_x(1, ring=nc.scalar) if NB > 1 else None
            Ws[0] = emit_weights(0)
            h1[0] = emit_L1(0, Ws[0], xsb0, kouter=True)
            if NB > 1:
                if run_of[1] not in Ws:
                    Ws[run_of[1]] = emit_weights(run_of[1])
                h1[1] = emit_L1(1, Ws[run_of[1]], x1)
            for b in range(NB):
                nxt = b + 2
                if nxt < NB:
                    g = run_of[nxt]
                    if g not in Ws:
                        Ws[g] = emit_weights(g)
                    ring = nc.sync if nxt == 2 else None
                    h1[nxt] = emit_L1(nxt, Ws[g], emit_x(nxt, ring=ring))
                emit_L23(b, Ws[run_of[b]], h1.pop(b))

    nc.compile()
    _program_cache[key] = nc
    return nc


# ---------------------------------------------------------------- host
def _execute(inputs, trace=False, trace_cores=None):
    graph = np.ascontiguousarray(inputs["graph"], dtype=np.float32)
    state = np.ascontiguousarray(inputs["state"], dtype=np.float32)
    next_state = np.ascontiguousarray(inputs["next_state"], dtype=np.float32)
    W1 = np.ascontiguousarray(inputs["W1"], dtype=np.float32)
    b1 = np.ascontiguousarray(inputs["b1"], dtype=np.float32)
    W2 = np.ascontiguousarray(inputs["W2"], dtype=np.float32)
    b2 = np.ascontiguousarray(inputs["b2"], dtype=np.float32)
    W3 = np.ascontiguousarray(inputs["W3"], dtype=np.float32)
    b3 = np.ascontiguousarray(inputs["b3"], dtype=np.float32)

    B = graph.shape[0]
    NF, IN, H = W1.shape
    C = W3.shape[2]
    assert IN == graph.shape[1] + state.shape[1] + next_state.shape[1]
    assert H % 128 == 0 and C <= 128
    INP = ((IN + 127) // 128) * 128
    KO1 = INP // 128

    out_full = np.zeros((B, C), dtype=np.float32)

    # --- route: last active factor per row
    mask = graph[:, :NF] == 1.0
    active = mask.any(axis=1)
    last = (NF - 1) - np.argmax(mask[:, ::-1], axis=1)
    if not active.any():
        return (out_full, None) if trace else out_full

    rows_by_e = [np.nonzero(active & (last == e))[0] for e in range(NF)]

    plan2 = _plan_runs([len(r) for r in rows_by_e])
    if plan2 is not None and sum(len(r) for r in plan2[0]) >= 3:
        return _execute_v2(plan2, rows_by_e, out_full, graph, state,
                           next_state, W1, b1, W2, b2, W3, b3,
                           KO1, H, C, trace, trace_cores)

    nblk = [(len(r) + BLK - 1) // BLK for r in rows_by_e]
    prof, expert_of = _make_plan(nblk)
    G, NB = len(prof), sum(prof)

    # --- pack rows into per-core block slots
    # rowmap[core] : int32 [NB, BLK], original row id or -1 (pad)
    rowmap = [np.full((NB, BLK), -1, dtype=np.int64) for _ in range(NCORES)]
    off = np.cumsum([0] + prof)  # run g occupies blocks [off[g], off[g+1])
    slots_by_e = {}
    for core in range(NCORES):
        for g in range(G):
            slots_by_e.setdefault(expert_of[core][g], []).append((core, g))
    for e in range(NF):
        rows = rows_by_e[e]
        if len(rows) == 0:
            continue
        pos = 0
        for core, g in slots_by_e.get(e, []):
            cap = prof[g] * BLK
            take = min(cap, len(rows) - pos)
            if take <= 0:
                break
            flat = rowmap[core][off[g]:off[g + 1]].reshape(-1)
            flat[:take] = rows[pos:pos + take]
            pos += take
        assert pos == len(rows), f"expert {e} rows not fully packed"

    # --- build per-core inputs
    x = np.concatenate([graph, state, next_state], axis=1)  # [B, IN]
    if INP != IN:
        x = np.concatenate([x, np.zeros((B, INP - IN), np.float32)], axis=1)
    xpad = np.concatenate([x, np.zeros((1, INP), np.float32)], axis=0)
    W1p = np.zeros((NF, INP, H), np.float32)
    W1p[:, :IN] = W1

    # Partition-major device layouts: [.., 128, KO, free] so every DMA
    # line is one contiguous 10-20KB run per partition.
    KO2 = H // 128
    W1pm = np.ascontiguousarray(
        W1p.reshape(NF, KO1, 128, H).transpose(0, 2, 1, 3).astype(NP_BF16))
    W2pm = np.ascontiguousarray(
        W2.reshape(NF, KO2, 128, H).transpose(0, 2, 1, 3).astype(NP_BF16))
    W3pm = np.ascontiguousarray(
        W3.reshape(NF, KO2, 128, C).transpose(0, 2, 1, 3).astype(NP_BF16))
    xpad = xpad.astype(NP_BF16)
    in_maps = []
    for core in range(NCORES):
        xb = xpad[rowmap[core].reshape(-1)]  # [NB*BLK, INP]; -1 -> zero row
        xb = np.ascontiguousarray(
            xb.reshape(NB, BLK, KO1, 128).transpose(0, 3, 2, 1))
        es = expert_of[core]
        in_maps.append({
            "xb": xb,
            "w1": W1pm[es],
            "w2": W2pm[es],
            "w3": W3pm[es],
            "b1": np.ascontiguousarray(b1[es]),
            "b2": np.ascontiguousarray(b2[es]),
            "b3": np.ascontiguousarray(b3[es]),
        })

    nc = _build_program(prof, KO1, KO2, H, C, BLK)
    kwargs = {}
    if trace:
        kwargs = dict(trace=True,
                      trace_cores=trace_cores or list(range(NCORES)))
    res = run_bass_kernel_spmd(nc, in_maps, list(range(NCORES)), **kwargs)

    # --- scatter back
    for core in range(NCORES):
        ob = np.asarray(res.results[core]["outb"])  # [NB, C, BLK]
        rows = ob.transpose(0, 2, 1).reshape(NB * BLK, C)
        ids = rowmap[core].reshape(-1)
        valid = ids >= 0
        out_full[ids[valid]] = rows[valid]

    return (out_full, res) if trace else out_full


def _execute_v2(plan2, rows_by_e, out_full, graph, state, next_state,
                W1, b1, W2, b2, W3, b3, KO1, H, C, trace, trace_cores):
    runs, expert_of, nrows_of = plan2
    G = len(runs)
    bsizes = [bs for r in runs for bs in r]
    run_of = [g for g, r in enumerate(runs) for _ in r]
    NB = len(bsizes)
    NF = W1.shape[0]
    B = graph.shape[0]
    IN = W1.shape[1]
    INP = KO1 * 128
    KO2 = H // 128

    x = np.concatenate([graph, state, next_state], axis=1)
    if INP != IN:
        x = np.concatenate([x, np.zeros((B, INP - IN), np.float32)], axis=1)
    xpad = np.concatenate(
        [x, np.zeros((1, INP), np.float32)], axis=0).astype(NP_BF16)

    W1p = np.zeros((NF, INP, H), np.float32)
    W1p[:, :IN] = W1
    W1pm = np.ascontiguousarray(
        W1p.reshape(NF, KO1, 128, H).transpose(0, 2, 1, 3).astype(NP_BF16))
    W2pm = np.ascontiguousarray(
        W2.reshape(NF, KO2, 128, H).transpose(0, 2, 1, 3).astype(NP_BF16))
    W3pm = np.ascontiguousarray(
        W3.reshape(NF, KO2, 128, C).transpose(0, 2, 1, 3).astype(NP_BF16))
    M1 = H // 128
    ball = np.zeros((NF, 128, 2 * M1 + 1), np.float32)
    ball[:, :, :M1] = b1.reshape(NF, M1, 128).transpose(0, 2, 1)
    ball[:, :, M1:2 * M1] = b2.reshape(NF, M1, 128).transpose(0, 2, 1)
    ball[:, :C, 2 * M1] = b3

    # deal each expert's rows across its (core, run) pieces in order
    ptr = [0] * NF
    in_maps = []
    rowmaps = []  # per core: list of per-block row-id arrays (-1 = pad)
    for core in range(NCORES):
        es = expert_of[core]  # one expert per run
        imap = {
            "w1": np.ascontiguousarray(W1pm[es]),
            "w2": np.ascontiguousarray(W2pm[es]),
            "w3": np.ascontiguousarray(W3pm[es]),
            "ball": np.ascontiguousarray(ball[es]),
        }
        rmap = []
        for g in range(G):
            e = es[g]
            take = nrows_of[core][g]
            ids_run = np.full(sum(runs[g]), -1, dtype=np.int64)
            if take:
                ids_run[:take] = rows_by_e[e][ptr[e]:ptr[e] + take]
                ptr[e] += take
            off = 0
            for bs in runs[g]:
                rmap.append(ids_run[off:off + bs])
                off += bs
        for b in range(NB):
            xb = xpad[rmap[b]]  # [bs, INP]; -1 -> zero row
            imap[f"xb{b}"] = np.ascontiguousarray(
                xb.reshape(bsizes[b], KO1, 128).transpose(2, 1, 0))
        in_maps.append(imap)
        rowmaps.append(rmap)
    for e in range(NF):
        assert ptr[e] == len(rows_by_e[e]), f"expert {e} rows not packed"

    nc = _build_program2(runs, KO1, KO2, H, C)
    kwargs = {}
    if trace:
        kwargs = dict(trace=True,
                      trace_cores=trace_cores or list(range(NCORES)))
    res = run_bass_kernel_spmd(nc, in_maps, list(range(NCORES)), **kwargs)

    for core in range(NCORES):
        for b in range(NB):
            ob = np.asarray(res.results[core][f"outb{b}"])  # [C, bs]
            ids = rowmaps[core][b]
            valid = ids >= 0
            if valid.any():
                out_full[ids[valid]] = ob.T[valid]

    return (out_full, res) if trace else out_full


def kernel(**inputs):
    return _execute(inputs)



# revision 27
# speedup vs baseline: 1.1897x; 1.0388x over previous
"""MoE-routed DIAYN discriminator kernel for 8 Trainium2 NeuronCores.

Reference semantics: x = concat([graph, state, next_state], -1); for each
row, run the 3-layer MLP of the LAST factor i<NF with graph[:, i]==1
(rows with no active factor output 0). The dense reference computes all
NF expert MLPs for every row; we instead route each row to exactly one
expert on the host, pack rows into 8 SPMD shards, and run one dense
per-expert MLP stream per core.

Sharding: rows are grouped by expert into BLK-row blocks. Every core
executes the same static "profile" of G runs (run g = prof[g] blocks);
each run uses one weight set, supplied per-core as data. A small host-side
search picks (G, prof) and an assignment of runs -> experts that covers
the actual per-expert block counts with minimal padding + weight traffic.

Device kernel (per run, per block, activations kept transposed [feat, row]):
  h1 = relu(W1^T x + b1); h2 = relu(W2^T h1 + b2); out = W3^T h2 + b3
matmuls run in bf16 (full rate on the PE, half the DMA of fp32);
PSUM accumulation, biases and the final output stay fp32.
"""

import ml_dtypes
import numpy as np

import concourse.bass as bass
import concourse.mybir as mybir
from concourse import bacc
from concourse.tile import TileContext
from concourse.bass_utils import run_bass_kernel_spmd

NCORES = 8
BLK = 272  # rows per matmul block; <=512 (PSUM bank)

F32 = mybir.dt.float32
BF16 = mybir.dt.bfloat16
NP_BF16 = ml_dtypes.bfloat16

# Rough per-core cost weights for the plan search (ns).
_COST_BLOCK = int(152 * (BLK / 2.4 + 3))  # PE ns per block (152 matmuls)
_COST_RUN = 12_000  # partially-exposed weight-set DMA per extra run

# v2 (variable-block) cost weights, from HW measurement: a bf16 matmul of
# N columns takes ~N/2.4 + 3 ns, 152 matmuls per block; a per-block
# weight-set DMA adds queue pressure and the set for block b prefetches
# during block b-2/b-1's compute.
_V2_NS_PER_ROW = 152 / 2.4
_V2_NS_PER_BLOCK = 152 * 3 + 1200
_V2_SET_DMA_NS = 13_000  # per-ring half weight set at ~200 GB/s

_program_cache = {}


# ---------------------------------------------------------------- planning
def _compositions(total, parts):
    """Non-increasing positive integer compositions of `total` into `parts`."""
    if parts == 1:
        yield (total,)
        return
    for first in range((total + parts - 1) // parts, total - parts + 2):
        for rest in _compositions(total - first, parts - 1):
            if rest[0] <= first:
                yield (first,) + rest


def _try_assign(demands, prof):
    """Greedy cover of per-expert block demands by the 8x-replicated profile.

    demands: list of (n_blocks, expert) sorted desc. Returns dict
    run_size -> list of experts (8 entries per profile slot of that size,
    padding slots filled with the largest expert) or None if infeasible.
    """
    runs = sorted([t for t in prof for _ in range(NCORES)], reverse=True)
    used = []  # (size, expert)
    for n, e in demands:
        rem = n
        while rem > 0:
            if not runs:
                return None
            # largest run <= rem, else smallest run (minimal overshoot)
            pick = None
            for i, s in enumerate(runs):
                if s <= rem:
                    pick = i
                    break
            if pick is None:
                pick = len(runs) - 1
            s = runs.pop(pick)
            used.append((s, e))
            rem -= s
    pad_expert = demands[0][1]
    for s in runs:
        used.append((s, pad_expert))
    by_size = {}
    for s, e in used:
        by_size.setdefault(s, []).append(e)
    return by_size


def _make_plan(nblk):
    """nblk: per-expert block counts. Returns (prof, expert_of[core][g])."""
    demands = sorted(
        [(n, e) for e, n in enumerate(nblk) if n > 0], reverse=True
    )
    total = sum(n for n, _ in demands)
    mincap = (total + NCORES - 1) // NCORES
    best = None
    for G in range(1, 9):
        for cap in range(mincap, mincap + 6):
            for prof in _compositions(cap, G):
                a = _try_assign(demands, prof)
                if a is None:
                    continue
                cost = cap * _COST_BLOCK + G * _COST_RUN
                if best is None or cost < best[0]:
                    best = (cost, prof, a)
    assert best is not None, "no feasible run plan found"
    _, prof, by_size = best
    queues = {s: list(es) for s, es in by_size.items()}
    expert_of = [[None] * len(prof) for _ in range(NCORES)]
    for g, s in enumerate(prof):
        for core in range(NCORES):
            expert_of[core][g] = queues[s].pop(0)
    return list(prof), expert_of


# ------------------------------------------------------------- planning v2
def _best_cover(rem, sizes, inv, limit):
    """Min-overshoot multiset of pieces covering `rem` rows.

    sizes: piece sizes desc; inv: available count per size; limit: prune
    bound on overshoot. Returns (overshoot, npieces, counts) or None.
    """
    best = [None]

    def rec(i, need, counts, used):
        if need <= 0:
            os = -need
            cand = (os, used, tuple(counts))
            if best[0] is None or cand < best[0]:
                best[0] = cand
            return
        if i == len(sizes):
            return
        if best[0] is not None and best[0][0] == 0 and used >= best[0][1]:
            return
        s = sizes[i]
        hi = min(inv[s], -(-need // s))
        for n in range(hi, -1, -1):
            if best[0] is not None and n * s - need > best[0][0] >= 0:
                continue
            counts.append(n)
            rec(i + 1, need - n * s, counts, used + n)
            counts.pop()

    rec(0, rem, [], 0)
    if best[0] is None or best[0][0] > limit:
        return None
    return best[0]


def _assign_rows(demands, bsizes):
    """Assign per-expert row demands to the 8x-replicated block slots.

    demands: [(rows, expert)] ascending. bsizes: per-core block sizes.
    Returns {expert: [(size, rows_used), ...]} or None.
    """
    sizes = sorted(set(bsizes), reverse=True)
    inv = {s: 8 * bsizes.count(s) for s in sizes}
    slack = 8 * sum(bsizes) - sum(r for r, _ in demands)
    out = {}
    for r, e in demands:
        got = _best_cover(r, sizes, inv, slack)
        if got is None:
            return None
        os, _, counts = got
        slack -= os
        pieces = []
        rem = r
        for s, n in zip(sizes, counts):
            inv[s] -= n
            for _ in range(n):
                take = min(rem, s)
                pieces.append((s, take))
                rem -= take
        out[e] = pieces
    return out


def _plan_runs(rows):
    """Pick a shared run/block structure + (core, run)->expert map.

    A run is a group of blocks sharing one weight set (one DMA per run).
    Returns (runs, expert_of[core][g], nrows_of[core][g]) or None; runs is
    a list of block-size lists, shared by all cores.
    """
    demands = sorted((r, e) for e, r in enumerate(rows) if r > 0)
    total = sum(r for r, _ in demands)
    if total == 0:
        return None

    S = list(range(512, 31, -32))
    cands = set()
    for r0 in (2, 3, 4):
        for t1 in S:
            cands.add((512 * r0, t1))
            for t2 in S:
                if t2 > t1:
                    continue
                cands.add((512 * r0, t1, t2))
                for t3 in S:
                    if t3 <= t2:
                        cands.add((512 * r0, t1, t2, t3))

    def blocks_of(caps):
        out = []
        for c in caps:
            while c > 0:
                out.append(min(512, c))
                c -= out[-1]
        return out

    def cost(caps):
        blks = blocks_of(caps)
        c = sum(caps) * _V2_NS_PER_ROW + len(blks) * _V2_NS_PER_BLOCK
        c += len(caps) * 4000  # per-run weight-set DMA pressure
        for i in range(1, len(caps)):
            c += max(0.0, _V2_SET_DMA_NS - caps[i - 1] * _V2_NS_PER_ROW)
        # sync-ring startup: W1(0) + x2 + W1(run1) (~44 us at measured
        # ring bw) must land before run 1's first L1 starts.
        j1 = -(-caps[0] // 512)
        l1d = [bs * 33.3 + 240 for bs in blks]
        l23d = [bs * 30.0 + 220 for bs in blks]
        t_run1 = sum(l1d[:j1]) + sum(l23d[:max(0, j1 - 2)])
        c += max(0.0, 44000 - t_run1)
        return c

    best = None
    for caps in sorted(cands, key=cost):
        cap8 = 8 * sum(caps)
        if not (total <= cap8 <= total + 6144):
            continue
        if best is not None and cost(caps) >= best[0]:
            continue
        assign = _assign_rows(demands, list(caps))
        if assign is None:
            assign = _assign_rows(demands[::-1], list(caps))
        if assign is None:
            continue
        best = (cost(caps), caps, assign)
    if best is None:
        return None
    _, caps, assign = best

    # queue of (expert, rows) pieces per cap; deal run-major to cores
    queues = {}
    for e, pieces in assign.items():
        for s, take in pieces:
            queues.setdefault(s, []).append((e, take))
    G = len(caps)
    expert_of = [[0] * G for _ in range(NCORES)]
    nrows_of = [[0] * G for _ in range(NCORES)]
    for g, s in enumerate(caps):
        for core in range(NCORES):
            q = queues.get(s) or []
            if q:
                e, take = q.pop(0)
            else:
                e, take = 0, 0
            expert_of[core][g] = e
            nrows_of[core][g] = take
    runs = []
    for c in caps:
        blks = []
        while c > 0:
            blks.append(min(512, c))
            c -= blks[-1]
        runs.append(blks)
    return runs, expert_of, nrows_of


# ---------------------------------------------------------------- device
def _build_program(prof, KO1, KO2, H, C, blk):
    """Build + compile the SPMD Bass program for a run profile."""
    key = (tuple(prof), KO1, KO2, H, C, blk)
    if key in _program_cache:
        return _program_cache[key]

    G = len(prof)
    NB = sum(prof)
    INP = KO1 * 128
    M1 = H // 128
    relu = mybir.ActivationFunctionType.Relu
    ident = mybir.ActivationFunctionType.Identity

    nc = bacc.Bacc("TRN2", target_bir_lowering=False, debug=False,
                   num_devices=NCORES)
    x_d = nc.dram_tensor("xb", [NB, 128, KO1, blk], BF16, kind="ExternalInput").ap()
    w1_d = nc.dram_tensor("w1", [G, 128, KO1, H], BF16, kind="ExternalInput").ap()
    w2_d = nc.dram_tensor("w2", [G, 128, KO2, H], BF16, kind="ExternalInput").ap()
    w3_d = nc.dram_tensor("w3", [G, 128, KO2, C], BF16, kind="ExternalInput").ap()
    b1_d = nc.dram_tensor("b1", [G, H], F32, kind="ExternalInput").ap()
    b2_d = nc.dram_tensor("b2", [G, H], F32, kind="ExternalInput").ap()
    b3_d = nc.dram_tensor("b3", [G, C], F32, kind="ExternalInput").ap()
    out_d = nc.dram_tensor("outb", [NB, C, blk], F32, kind="ExternalOutput").ap()

    runs = []
    for g, T in enumerate(prof):
        runs += [g] * T

    with TileContext(nc) as tc:
        with (
            tc.tile_pool(name="w", bufs=2) as wpool,
            tc.tile_pool(name="x", bufs=2) as xpool,
            tc.tile_pool(name="h1", bufs=3) as h1pool,
            tc.tile_pool(name="h2", bufs=1) as h2pool,
            tc.tile_pool(name="o", bufs=2) as opool,
            tc.tile_pool(name="ps", bufs=8, space="PSUM") as pspool,
        ):
            def emit_weights(g, x_first=None, x_hook=None, x_hook2=None):
                # Biases first (tiny, needed by the first relu). W1 as
                # per-k-tile chunks so block-0's k-outer L1 can consume
                # them as they arrive; W2 as halves (needed later).
                w1ch = []
                b1sb = b2sb = b3sb = None
                for k in range(KO1):
                    if x_first is not None:
                        nc.sync.dma_start(x_first[0][:, k, :],
                                          x_first[1][:, k, :])
                    wt = wpool.tile([128, H], BF16, tag=f"w1k{k}")
                    nc.sync.dma_start(wt[:], w1_d[g, :, k, :])
                    w1ch.append(wt)
                    if k == 0:
                        # Biases after the first chunk pair (PE can start)
                        # but well before the first relu needs them.
                        b1sb = wpool.tile([128, M1], F32, tag="b1")
                        nc.sync.dma_start(
                            b1sb[:],
                            b1_d[g].rearrange("(m p) -> p m", p=128))
                        b2sb = wpool.tile([128, M1], F32, tag="b2")
                        nc.sync.dma_start(
                            b2sb[:],
                            b2_d[g].rearrange("(m p) -> p m", p=128))
                        b3sb = wpool.tile([C, 1], F32, tag="b3")
                        nc.sync.dma_start(b3sb[:], b3_d[g][:, None])
                if x_hook is not None:
                    x_hook()
                KH2 = KO2 // 2
                w2a = wpool.tile([128, KH2, H], BF16, tag="w2a")
                nc.sync.dma_start(w2a[:], w2_d[g, :, :KH2, :])
                if x_hook2 is not None:
                    x_hook2()
                w2b = wpool.tile([128, KO2 - KH2, H], BF16, tag="w2b")
                nc.sync.dma_start(w2b[:], w2_d[g, :, KH2:, :])
                w3sb = wpool.tile([128, KO2, C], BF16, tag="w3")
                nc.sync.dma_start(w3sb[:], w3_d[g])

                def w2(k):
                    return w2a[:, k, :] if k < KH2 else w2b[:, k - KH2, :]

                return dict(w1=lambda k: w1ch[k], w2=w2, w3=w3sb,
                            b1=b1sb, b2=b2sb, b3=b3sb)

            def emit_x(b):
                # x blocks ride the second HWDGE ring (scalar), parallel
                # to the weight stream on sync.
                xsb = xpool.tile([128, KO1, blk], BF16, tag="x")
                nc.scalar.dma_start(xsb[:], x_d[b])
                return xsb

            def emit_L1(W, xsb, kouter=False):
                h1sb = h1pool.tile([128, KO2, blk], BF16, tag="h1")
                if kouter:
                    # All 8 PSUM banks accumulate in parallel; each W1
                    # chunk is fully consumed on arrival (startup mode).
                    pss = [pspool.tile([128, blk], F32, tag="ps",
                                       name=f"ps_ko{m}")
                           for m in range(M1)]
                    for k in range(KO1):
                        for m in range(M1):
                            nc.tensor.matmul(
                                pss[m][:],
                                W["w1"](k)[:, m * 128:(m + 1) * 128],
                                xsb[:, k, :],
                                start=(k == 0), stop=(k == KO1 - 1))
                    for m in range(M1):
                        nc.vector.tensor_scalar(
                            h1sb[:, m, :], pss[m][:], W["b1"][:, m:m + 1],
                            0.0, mybir.AluOpType.add, mybir.AluOpType.max)
                    return h1sb
                for m in range(M1):
                    ps = pspool.tile([128, blk], F32, tag="ps")
                    for k in range(KO1):
                        nc.tensor.matmul(
                            ps[:],
                            W["w1"](k)[:, m * 128:(m + 1) * 128],
                            xsb[:, k, :],
                            start=(k == 0), stop=(k == KO1 - 1))
                    nc.vector.tensor_scalar(
                        h1sb[:, m, :], ps[:], W["b1"][:, m:m + 1], 0.0,
                        mybir.AluOpType.add, mybir.AluOpType.max)
                return h1sb

            def emit_L23(b, W, h1sb):
                h2sb = h2pool.tile([128, KO2, blk], BF16, tag="h2")
                for m in range(M1):
                    ps = pspool.tile([128, blk], F32, tag="ps")
                    for k in range(KO2):
                        nc.tensor.matmul(
                            ps[:],
                            W["w2"](k)[:, m * 128:(m + 1) * 128],
                            h1sb[:, k, :],
                            start=(k == 0), stop=(k == KO2 - 1))
                    nc.scalar.activation(
                        h2sb[:, m, :], ps[:], relu, bias=W["b2"][:, m:m + 1])
                ps3 = pspool.tile([128, blk], F32, tag="ps")
                for k in range(KO2):
                    nc.tensor.matmul(
                        ps3[:C, :],
                        W["w3"][:, k, :],
                        h2sb[:, k, :],
                        start=(k == 0), stop=(k == KO2 - 1))
                osb = opool.tile([C, blk], F32, tag="o")
                nc.scalar.activation(
                    osb[:], ps3[:C, :], ident, bias=W["b3"][:, 0:1])
                nc.gpsimd.dma_start(out_d[b], osb[:])

            # Software pipeline, depth 2: L1 of blocks b+1/b+2 are
            # emitted before L2/L3 of block b, so weight-set DMAs and
            # ACT latency never drain the PE (esp. during the initial
            # HBM-bound weight load).
            Ws = {}
            h1 = {}

            xpre = {}

            def emit_front(b):
                g = runs[b]
                if g not in Ws:
                    Ws[g] = emit_weights(g)
                h1[b] = emit_L1(Ws[g], xpre.pop(b) if b in xpre
                                else emit_x(b))

            # Startup: x0/x1 lead the scalar ring while weights
            # stream on sync; L1(0)/L1(1) are emitted before L2(0) so
            # the PE has work during the HBM-bound weight load. Steady
            # state keeps L1 two blocks ahead of L2/L3.
            def emit_x_sync(b):
                xsb = xpool.tile([128, KO1, blk], BF16, tag="x")
                nc.sync.dma_start(xsb[:], x_d[b])
                return xsb

            # Startup: everything for the first ~3 blocks rides the sync
            # ring in consumption order (x0 interleaved with W1 chunks,
            # then x1, W2a, x2, W2b); block 0's L1 runs k-outer so each
            # W1 chunk is consumed on arrival.
            g0 = runs[0]
            if prof[0] >= 3:
                xsb0 = xpool.tile([128, KO1, blk], BF16, tag="x")
                xs = {}
                def _x12():
                    xs[1] = emit_x_sync(1)
                    xs[2] = emit_x_sync(2)

                Ws[g0] = emit_weights(g0, x_first=(xsb0, x_d[0]),
                                      x_hook=_x12)
                # x3/x4 ride the idle SWDGE ring: the scalar ring's
                # issue slot is blocked behind early L2-relus right at
                # the prologue->steady transition.
                for bb in (3, 4):
                    if bb < NB:
                        xp = xpool.tile([128, KO1, blk], BF16, tag="x",
                                        name=f"xpre{bb}")
                        nc.gpsimd.dma_start(xp[:], x_d[bb])
                        xpre[bb] = xp
                h1[0] = emit_L1(Ws[g0], xsb0, kouter=True)
                h1[1] = emit_L1(Ws[g0], xs[1])
                h1[2] = emit_L1(Ws[g0], xs[2])
                emitted = 2
            elif NB > 1 and runs[1] == g0:
                xsb0 = xpool.tile([128, KO1, blk], BF16, tag="x")
                xs1 = []
                Ws[g0] = emit_weights(g0, x_first=(xsb0, x_d[0]),
                                      x_hook=lambda: xs1.append(emit_x(1)))
                h1[0] = emit_L1(Ws[g0], xsb0, kouter=True)
                h1[1] = emit_L1(Ws[g0], xs1[0])
                emitted = 1
            else:
                emit_front(0)
                emitted = 0
            for b in range(NB):
                for nxt in range(emitted + 1, min(b + 3, NB)):
                    emit_front(nxt)
                    emitted = nxt
                if b + 4 < NB and runs[b + 4] not in Ws:
                    Ws[runs[b + 4]] = emit_weights(runs[b + 4])
                emit_L23(b, Ws[runs[b]], h1.pop(b))

    nc.compile()
    _program_cache[key] = nc
    return nc


# ---------------------------------------------------------------- device v2
def _build_program2(runs, KO1, KO2, H, C):
    """Variable-block SPMD program. Each run (group of blocks) shares one
    weight set, supplied per-core as data. Rings: W1+biases (+x1) on sync,
    W2/W3 on scalar, x blocks + outputs on gpsimd (SWDGE)."""
    key = ("v3", tuple(tuple(r) for r in runs), KO1, KO2, H, C)
    if key in _program_cache:
        return _program_cache[key]

    G = len(runs)
    blocks = [bs for r in runs for bs in r]
    run_of = [g for g, r in enumerate(runs) for _ in r]
    NB = len(blocks)
    M1 = H // 128
    KH2 = KO2 // 2
    relu = mybir.ActivationFunctionType.Relu
    ident = mybir.ActivationFunctionType.Identity

    nc = bacc.Bacc("TRN2", target_bir_lowering=False, debug=False,
                   num_devices=NCORES)
    x_ds = [nc.dram_tensor(f"xb{b}", [128, KO1, bs], BF16,
                           kind="ExternalInput").ap()
            for b, bs in enumerate(blocks)]
    out_ds = [nc.dram_tensor(f"outb{b}", [C, bs], F32,
                             kind="ExternalOutput").ap()
              for b, bs in enumerate(blocks)]
    w1_d = nc.dram_tensor("w1", [G, 128, KO1, H], BF16,
                          kind="ExternalInput").ap()
    w2_d = nc.dram_tensor("w2", [G, 128, KO2, H], BF16,
                          kind="ExternalInput").ap()
    w3_d = nc.dram_tensor("w3", [G, 128, KO2, C], BF16,
                          kind="ExternalInput").ap()
    # biases pre-packed partition-major on the host: cols [0,M1) = b1,
    # [M1,2*M1) = b2, col 2*M1 = b3 (first C partitions). One contiguous
    # DMA per set -- tiny strided bias transfers wedge the sync queue for
    # ~6 us each and starve the startup W1 stream.
    ball_d = nc.dram_tensor("ball", [G, 128, 2 * M1 + 1], F32,
                            kind="ExternalInput").ap()

    with TileContext(nc) as tc:
        with (
            tc.tile_pool(name="w", bufs=2) as wpool,
            tc.tile_pool(name="x", bufs=2) as xpool,
            tc.tile_pool(name="h1", bufs=3) as h1pool,
            tc.tile_pool(name="h2", bufs=1) as h2pool,
            tc.tile_pool(name="o", bufs=2) as opool,
            tc.tile_pool(name="ps", bufs=8, space="PSUM") as pspool,
        ):
            def emit_w1(g, x_interleave=None):
                # W1 as per-k chunks + packed biases. With x_interleave
                # (startup), chunk k and x-slice k alternate between the
                # two HWDGE rings so W1(0)+x0 stream at ~2x one ring's
                # bandwidth, and the k-outer block consumes them on
                # arrival (subtile deps track the per-slice DMAs).
                w1ch = []
                ball = None
                for k in range(KO1):
                    ring = nc.scalar if (x_interleave and k % 2) else nc.sync
                    wt = wpool.tile([128, H], BF16, tag=f"w1k{k}")
                    ring.dma_start(wt[:], w1_d[g, :, k, :])
                    w1ch.append(wt)
                    if x_interleave is not None:
                        xsb, xd = x_interleave
                        ring.dma_start(xsb[:, k, :], xd[:, k, :])
                    if k == 0:
                        ball = wpool.tile([128, 2 * M1 + 1], F32,
                                          tag="ball", bufs=3)
                        nc.sync.dma_start(ball[:], ball_d[g])
                return dict(w1=lambda kk: w1ch[kk],
                            b1=ball[:, 0:M1], b2=ball[:, M1:2 * M1],
                            b3=ball[:C, 2 * M1:2 * M1 + 1])

            def emit_w23(g, W, ring=None):
                # W2 halves and W3 (bufs=3 so the slot-free waits are
                # already resolved at issue time and never park the
                # issuing engine between PSUM-draining relus).
                ring = ring or nc.scalar
                w2a = wpool.tile([128, KH2, H], BF16, tag="w2a", bufs=3)
                ring.dma_start(w2a[:], w2_d[g, :, :KH2, :])
                w2b = wpool.tile([128, KO2 - KH2, H], BF16, tag="w2b",
                                 bufs=3)
                ring.dma_start(w2b[:], w2_d[g, :, KH2:, :])
                w3sb = wpool.tile([128, KO2, C], BF16, tag="w3", bufs=3)
                ring.dma_start(w3sb[:], w3_d[g])

                def w2(k):
                    return w2a[:, k, :] if k < KH2 else w2b[:, k - KH2, :]

                W["w2"] = w2
                W["w3"] = w3sb
                return W

            def emit_weights(g):
                return emit_w23(g, emit_w1(g))

            def emit_x(b, ring=None):
                bs = blocks[b]
                xsb = xpool.tile([128, KO1, bs], BF16, tag="x", bufs=3)
                (ring or nc.gpsimd).dma_start(xsb[:], x_ds[b][:])
                return xsb

            def emit_L1(b, W, xsb, kouter=False):
                bs = blocks[b]
                h1sb = h1pool.tile([128, KO2, bs], BF16, tag="h1")
                if kouter:
                    # Startup: all 8 PSUM banks accumulate in parallel so
                    # each W1 chunk is fully consumed as it arrives.
                    pss = [pspool.tile([128, bs], F32, tag="ps",
                                       name=f"ps_ko{m}")
                           for m in range(M1)]
                    for k in range(KO1):
                        for m in range(M1):
                            nc.tensor.matmul(
                                pss[m][:],
                                W["w1"](k)[:, m * 128:(m + 1) * 128],
                                xsb[:, k, :],
                                start=(k == 0), stop=(k == KO1 - 1))
                    for m in range(M1):
                        nc.vector.tensor_scalar(
                            h1sb[:, m, :], pss[m][:], W["b1"][:, m:m + 1],
                            0.0, mybir.AluOpType.add, mybir.AluOpType.max)
                    return h1sb
                for m in range(M1):
                    ps = pspool.tile([128, bs], F32, tag="ps")
                    for k in range(KO1):
                        nc.tensor.matmul(
                            ps[:],
                            W["w1"](k)[:, m * 128:(m + 1) * 128],
                            xsb[:, k, :],
                            start=(k == 0), stop=(k == KO1 - 1))
                    nc.vector.tensor_scalar(
                        h1sb[:, m, :], ps[:], W["b1"][:, m:m + 1], 0.0,
                        mybir.AluOpType.add, mybir.AluOpType.max)
                return h1sb

            def emit_L23(b, W, h1sb):
                bs = blocks[b]
                h2sb = h2pool.tile([128, KO2, bs], BF16, tag="h2")
                for m in range(M1):
                    ps = pspool.tile([128, bs], F32, tag="ps")
                    for k in range(KO2):
                        nc.tensor.matmul(
                            ps[:],
                            W["w2"](k)[:, m * 128:(m + 1) * 128],
                            h1sb[:, k, :],
                            start=(k == 0), stop=(k == KO2 - 1))
                    nc.scalar.activation(
                        h2sb[:, m, :], ps[:], relu, bias=W["b2"][:, m:m + 1])
                ps3 = pspool.tile([128, bs], F32, tag="ps")
                for k in range(KO2):
                    nc.tensor.matmul(
                        ps3[:C, :],
                        W["w3"][:, k, :],
                        h2sb[:, k, :],
                        start=(k == 0), stop=(k == KO2 - 1))
                osb = opool.tile([C, bs], F32, tag="o")
                nc.scalar.activation(
                    osb[:], ps3[:C, :], ident, bias=W["b3"][:, 0:1])
                nc.gpsimd.dma_start(out_ds[b][:], osb[:])

            # Two-deep software pipeline: L1 leads L23 by two blocks and a
            # run's weight set is DMA-queued as soon as one of its blocks
            # enters the lookahead. Startup: W1(0) chunks + x0 slices
            # alternate across both HWDGE rings (block 0 is HBM-bound:
            # ~227 GB/s needed vs ~150 per ring), x1 streams in k-slices
            # right behind them, and W2/W3(0) (not needed until L23(0))
            # is deferred so it stays off the critical window. x2 rides
            # sync, later x blocks ride gpsimd.
            h1 = {}
            Ws = {}
            xsb0 = xpool.tile([128, KO1, blocks[0]], BF16, tag="x",
                              bufs=3)
            Ws[0] = emit_w1(0, x_interleave=(xsb0, x_ds[0]))
            if NB > 1:
                x1sb = xpool.tile([128, KO1, blocks[1]], BF16, tag="x",
                                  bufs=3)
                for k in range(KO1):
                    ring = nc.scalar if k % 2 else nc.sync
                    ring.dma_start(x1sb[:, k, :], x_ds[1][:, k, :])
            h1[0] = emit_L1(0, Ws[0], xsb0, kouter=True)
            emit_w23(0, Ws[0])
            if NB > 1:
                g1 = run_of[1]
                if g1 not in Ws:
                    Ws[g1] = emit_w23(g1, emit_w1(g1))
                h1[1] = emit_L1(1, Ws[g1], x1sb)
            for b in range(NB):
                nxt = b + 2
                if nxt < NB:
                    g = run_of[nxt]
                    if g not in Ws:
                        Ws[g] = emit_w23(g, emit_w1(g))
                    ring = nc.sync if nxt == 2 else None
                    h1[nxt] = emit_L1(nxt, Ws[g], emit_x(nxt, ring=ring))
                emit_L23(b, Ws[run_of[b]], h1.pop(b))

    nc.compile()
    _program_cache[key] = nc
    return nc


# ---------------------------------------------------------------- host
def _execute(inputs, trace=False, trace_cores=None):
    graph = np.ascontiguousarray(inputs["graph"], dtype=np.float32)
    state = np.ascontiguousarray(inputs["state"], dtype=np.float32)
    next_state = np.ascontiguousarray(inputs["next_state"], dtype=np.float32)
    W1 = np.ascontiguousarray(inputs["W1"], dtype=np.float32)
    b1 = np.ascontiguousarray(inputs["b1"], dtype=np.float32)
    W2 = np.ascontiguousarray(inputs["W2"], dtype=np.float32)
    b2 = np.ascontiguousarray(inputs["b2"], dtype=np.float32)
    W3 = np.ascontiguousarray(inputs["W3"], dtype=np.float32)
    b3 = np.ascontiguousarray(inputs["b3"], dtype=np.float32)

    B = graph.shape[0]
    NF, IN, H = W1.shape
    C = W3.shape[2]
    assert IN == graph.shape[1] + state.shape[1] + next_state.shape[1]
    assert H % 128 == 0 and C <= 128
    INP = ((IN + 127) // 128) * 128
    KO1 = INP // 128

    out_full = np.zeros((B, C), dtype=np.float32)

    # --- route: last active factor per row
    mask = graph[:, :NF] == 1.0
    active = mask.any(axis=1)
    last = (NF - 1) - np.argmax(mask[:, ::-1], axis=1)
    if not active.any():
        return (out_full, None) if trace else out_full

    rows_by_e = [np.nonzero(active & (last == e))[0] for e in range(NF)]

    plan2 = _plan_runs([len(r) for r in rows_by_e])
    if plan2 is not None and sum(len(r) for r in plan2[0]) >= 3:
        return _execute_v2(plan2, rows_by_e, out_full, graph, state,
                           next_state, W1, b1, W2, b2, W3, b3,
                           KO1, H, C, trace, trace_cores)

    nblk = [(len(r) + BLK - 1) // BLK for r in rows_by_e]
    prof, expert_of = _make_plan(nblk)
    G, NB = len(prof), sum(prof)

    # --- pack rows into per-core block slots
    # rowmap[core] : int32 [NB, BLK], original row id or -1 (pad)
    rowmap = [np.full((NB, BLK), -1, dtype=np.int64) for _ in range(NCORES)]
    off = np.cumsum([0] + prof)  # run g occupies blocks [off[g], off[g+1])
    slots_by_e = {}
    for core in range(NCORES):
        for g in range(G):
            slots_by_e.setdefault(expert_of[core][g], []).append((core, g))
    for e in range(NF):
        rows = rows_by_e[e]
        if len(rows) == 0:
            continue
        pos = 0
        for core, g in slots_by_e.get(e, []):
            cap = prof[g] * BLK
            take = min(cap, len(rows) - pos)
            if take <= 0:
                break
            flat = rowmap[core][off[g]:off[g + 1]].reshape(-1)
            flat[:take] = rows[pos:pos + take]
            pos += take
        assert pos == len(rows), f"expert {e} rows not fully packed"

    # --- build per-core inputs
    x = np.concatenate([graph, state, next_state], axis=1)  # [B, IN]
    if INP != IN:
        x = np.concatenate([x, np.zeros((B, INP - IN), np.float32)], axis=1)
    xpad = np.concatenate([x, np.zeros((1, INP), np.float32)], axis=0)
    W1p = np.zeros((NF, INP, H), np.float32)
    W1p[:, :IN] = W1

    # Partition-major device layouts: [.., 128, KO, free] so every DMA
    # line is one contiguous 10-20KB run per partition.
    KO2 = H // 128
    W1pm = np.ascontiguousarray(
        W1p.reshape(NF, KO1, 128, H).transpose(0, 2, 1, 3).astype(NP_BF16))
    W2pm = np.ascontiguousarray(
        W2.reshape(NF, KO2, 128, H).transpose(0, 2, 1, 3).astype(NP_BF16))
    W3pm = np.ascontiguousarray(
        W3.reshape(NF, KO2, 128, C).transpose(0, 2, 1, 3).astype(NP_BF16))
    xpad = xpad.astype(NP_BF16)
    in_maps = []
    for core in range(NCORES):
        xb = xpad[rowmap[core].reshape(-1)]  # [NB*BLK, INP]; -1 -> zero row
        xb = np.ascontiguousarray(
            xb.reshape(NB, BLK, KO1, 128).transpose(0, 3, 2, 1))
        es = expert_of[core]
        in_maps.append({
            "xb": xb,
            "w1": W1pm[es],
            "w2": W2pm[es],
            "w3": W3pm[es],
            "b1": np.ascontiguousarray(b1[es]),
            "b2": np.ascontiguousarray(b2[es]),
            "b3": np.ascontiguousarray(b3[es]),
        })

    nc = _build_program(prof, KO1, KO2, H, C, BLK)
    kwargs = {}
    if trace:
        kwargs = dict(trace=True,
                      trace_cores=trace_cores or list(range(NCORES)))
    res = run_bass_kernel_spmd(nc, in_maps, list(range(NCORES)), **kwargs)

    # --- scatter back
    for core in range(NCORES):
        ob = np.asarray(res.results[core]["outb"])  # [NB, C, BLK]
        rows = ob.transpose(0, 2, 1).reshape(NB * BLK, C)
        ids = rowmap[core].reshape(-1)
        valid = ids >= 0
        out_full[ids[valid]] = rows[valid]

    return (out_full, res) if trace else out_full


def _execute_v2(plan2, rows_by_e, out_full, graph, state, next_state,
                W1, b1, W2, b2, W3, b3, KO1, H, C, trace, trace_cores):
    runs, expert_of, nrows_of = plan2
    G = len(runs)
    bsizes = [bs for r in runs for bs in r]
    run_of = [g for g, r in enumerate(runs) for _ in r]
    NB = len(bsizes)
    NF = W1.shape[0]
    B = graph.shape[0]
    IN = W1.shape[1]
    INP = KO1 * 128
    KO2 = H // 128

    x = np.concatenate([graph, state, next_state], axis=1)
    if INP != IN:
        x = np.concatenate([x, np.zeros((B, INP - IN), np.float32)], axis=1)
    xpad = np.concatenate(
        [x, np.zeros((1, INP), np.float32)], axis=0).astype(NP_BF16)

    W1p = np.zeros((NF, INP, H), np.float32)
    W1p[:, :IN] = W1
    W1pm = np.ascontiguousarray(
        W1p.reshape(NF, KO1, 128, H).transpose(0, 2, 1, 3).astype(NP_BF16))
    W2pm = np.ascontiguousarray(
        W2.reshape(NF, KO2, 128, H).transpose(0, 2, 1, 3).astype(NP_BF16))
    W3pm = np.ascontiguousarray(
        W3.reshape(NF, KO2, 128, C).transpose(0, 2, 1, 3).astype(NP_BF16))
    M1 = H // 128
    ball = np.zeros((NF, 128, 2 * M1 + 1), np.float32)
    ball[:, :, :M1] = b1.reshape(NF, M1, 128).transpose(0, 2, 1)
    ball[:, :, M1:2 * M1] = b2.reshape(NF, M1, 128).transpose(0, 2, 1)
    ball[:, :C, 2 * M1] = b3

    # deal each expert's rows across its (core, run) pieces in order
    ptr = [0] * NF
    in_maps = []
    rowmaps = []  # per core: list of per-block row-id arrays (-1 = pad)
    for core in range(NCORES):
        es = expert_of[core]  # one expert per run
        imap = {
            "w1": np.ascontiguousarray(W1pm[es]),
            "w2": np.ascontiguousarray(W2pm[es]),
            "w3": np.ascontiguousarray(W3pm[es]),
            "ball": np.ascontiguousarray(ball[es]),
        }
        rmap = []
        for g in range(G):
            e = es[g]
            take = nrows_of[core][g]
            ids_run = np.full(sum(runs[g]), -1, dtype=np.int64)
            if take:
                ids_run[:take] = rows_by_e[e][ptr[e]:ptr[e] + take]
                ptr[e] += take
            off = 0
            for bs in runs[g]:
                rmap.append(ids_run[off:off + bs])
                off += bs
        for b in range(NB):
            xb = xpad[rmap[b]]  # [bs, INP]; -1 -> zero row
            imap[f"xb{b}"] = np.ascontiguousarray(
                xb.reshape(bsizes[b], KO1, 128).transpose(2, 1, 0))
        in_maps.append(imap)
        rowmaps.append(rmap)
    for e in range(NF):
        assert ptr[e] == len(rows_by_e[e]), f"expert {e} rows not packed"

    nc = _build_program2(runs, KO1, KO2, H, C)
    kwargs = {}
    if trace:
        kwargs = dict(trace=True,
                      trace_cores=trace_cores or list(range(NCORES)))
    res = run_bass_kernel_spmd(nc, in_maps, list(range(NCORES)), **kwargs)

    for core in range(NCORES):
        for b in range(NB):
            ob = np.asarray(res.results[core][f"outb{b}"])  # [C, bs]
            ids = rowmaps[core][b]
            valid = ids >= 0
            if valid.any():
                out_full[ids[valid]] = ob.T[valid]

    return (out_full, res) if trace else out_full


def kernel(**inputs):
    return _execute(inputs)

